# revision 9
# baseline (speedup 1.0000x reference)
"""GCN encoder (2-layer, PyG GCNConv w/ self-loops + symmetric norm) on 8 trn2 cores.

Math: out = D^-1/2 (A+I) D^-1/2 X W + b per layer, done as
  x' = dis * x   (row scale, dis = deg^-1/2)
  agg[d] = sum_{edges s->d} x'[s]        (gather + segment-sum via PE matmul vs one-hot S)
  y[d] = f(dis[d] * agg[d] @ W + b)      (f = relu for layer 1)
Sharding: destinations row-sharded across 8 cores; x' replicated via
redundant prep (layer 1) / AllGather (layer 2).
"""

import sys

sys.path.insert(0, "/opt/trn_rl_repo")

import numpy as np
import ml_dtypes

BF16 = ml_dtypes.bfloat16

N_NODES = 50000
N_EDGES = 800000
D = 128
P = 8


def _sizes(n):
    rpc = -(-n // (P * 128)) * 128  # rows per core, multiple of 128
    npad = rpc * P
    b = rpc // 128  # dest blocks per core
    half = npad // 2
    nt = npad // 128
    # group size: largest divisor of b that is <= 8
    gs = 1
    for d_ in range(1, 9):
        if b % d_ == 0:
            gs = d_
    return rpc, npad, b, half, nt, gs


def plan(edge_index, n=N_NODES):
    """Host-side integer preprocessing: degrees, per-core padded edge slots,
    gather index tiles, relative-destination tiles."""
    rpc, npad, b, half, nt, gs = _sizes(n)
    src = edge_index[0].astype(np.int64)
    dst = edge_index[1].astype(np.int64)
    loops = np.arange(n, dtype=np.int64)
    allsrc = np.concatenate([src, loops])
    alldst = np.concatenate([dst, loops])

    deg = np.bincount(alldst, minlength=n).astype(np.float32)
    deg_pad = np.ones(npad, dtype=np.float32)
    deg_pad[:n] = deg
    deg_t = np.ascontiguousarray(deg_pad.reshape(nt, 128).T)  # [128, nt]

    core = alldst // rpc
    dloc = alldst - core * rpc
    blk = dloc >> 7
    drel = (dloc & 127).astype(np.float32)
    hi = (allsrc >= half).astype(np.int64)

    key = (core * b + blk) * 2 + hi
    nkeys = P * b * 2
    counts = np.bincount(key, minlength=nkeys)
    k2 = max(1, int(-(-counts.max() // 128)))
    slots_per = k2 * 128

    order = np.argsort(key, kind="stable")
    key_s = key[order]
    src_s = allsrc[order]
    drel_s = drel[order]
    run_start = np.zeros(nkeys, dtype=np.int64)
    np.cumsum(counts[:-1], out=run_start[1:])
    rank = np.arange(key_s.size, dtype=np.int64) - run_start[key_s]
    pos = key_s * slots_per + rank

    idx_slots = np.zeros(nkeys * slots_per, dtype=np.int16)
    drel_slots = np.full(nkeys * slots_per, -1.0, dtype=np.float32)
    idx_slots[pos] = (src_s - hi[order] * half).astype(np.int16)
    drel_slots[pos] = drel_s

    # [P, b, 2, k2*128]
    idx_slots = idx_slots.reshape(P, b, 2, slots_per)
    drel_slots = drel_slots.reshape(P, b, 2, slots_per)

    g = b // gs
    L = gs * k2 * 128  # idxs per gather call
    l16 = L // 16

    def make_idx(core_slots, h):
        # core_slots: [b, 2, k2*128] -> per group g: [gs*k2*128] block-major
        seq = core_slots[:, h, :].reshape(g, L)  # [g, L]
        tile = seq.reshape(g, l16, 16).transpose(0, 2, 1)  # [g, 16, l16]
        tile = np.tile(tile, (1, 8, 1))  # [g, 128, l16]
        return np.ascontiguousarray(tile.transpose(1, 0, 2).reshape(128, g * l16))

    per_core = []
    for c in range(P):
        idx_lo = make_idx(idx_slots[c], 0)
        idx_hi = make_idx(idx_slots[c], 1)
        # drel device layout [128, b*2*k2]: col (bb*2k2 + k) row p =
        #   drel_slots[c, bb, k<k2?0:1, (k%k2)*128 + p]
        dr = drel_slots[c].reshape(b, 2 * k2, 128).transpose(2, 0, 1)
        dr = np.ascontiguousarray(dr.reshape(128, b * 2 * k2)).astype(BF16)
        deg_own = np.ascontiguousarray(deg_t[:, c * b : (c + 1) * b])
        per_core.append(
            {"idx_lo": idx_lo, "idx_hi": idx_hi, "drel": dr, "deg_own": deg_own}
        )

    iota = np.tile(np.arange(128, dtype=np.float32), (128, 1)).astype(BF16)
    ident = np.eye(128, dtype=np.float32)
    return {
        "sizes": (rpc, npad, b, half, nt, gs, g, k2, L, l16),
        "deg_t": deg_t,
        "per_core": per_core,
        "iota": iota,
        "ident_bf": ident.astype(BF16),
        "ident_f32": ident,
    }


def build_program(pl, stage="full"):
    import concourse.mybir as mybir
    from concourse.bacc import Bacc
    from concourse.tile import TileContext

    rpc, npad, b, half, nt, gs, g, k2, L, l16 = pl["sizes"]
    f32 = mybir.dt.float32
    bf16 = mybir.dt.bfloat16
    i16 = mybir.dt.int16
    AF = mybir.ActivationFunctionType
    OP = mybir.AluOpType

    nc = Bacc(num_devices=P)

    x_in = nc.declare_dram_parameter("x", [npad, D], f32, isOutput=False)
    degt_in = nc.declare_dram_parameter("deg_t", [128, nt], f32, isOutput=False)
    dego_in = nc.declare_dram_parameter("deg_own", [128, b], f32, isOutput=False)
    w1_in = nc.declare_dram_parameter("W1", [D, D], f32, isOutput=False)
    b1_in = nc.declare_dram_parameter("b1", [D, 1], f32, isOutput=False)
    w2_in = nc.declare_dram_parameter("W2", [D, D], f32, isOutput=False)
    b2_in = nc.declare_dram_parameter("b2", [D, 1], f32, isOutput=False)
    iota_in = nc.declare_dram_parameter("iota", [128, 128], bf16, isOutput=False)
    identb_in = nc.declare_dram_parameter("ident_bf", [128, 128], bf16, isOutput=False)
    identf_in = nc.declare_dram_parameter("ident_f32", [128, 128], f32, isOutput=False)
    idxlo_in = nc.declare_dram_parameter("idx_lo", [128, g * l16], i16, isOutput=False)
    idxhi_in = nc.declare_dram_parameter("idx_hi", [128, g * l16], i16, isOutput=False)
    drel_in = nc.declare_dram_parameter("drel", [128, b * 2 * k2], bf16, isOutput=False)
    out = nc.declare_dram_parameter("out", [rpc, D], f32, isOutput=True)

    x1 = nc.dram_tensor("x1", [npad, D], bf16)
    x2own = nc.dram_tensor("x2own", [rpc, D], bf16)
    x2full = nc.dram_tensor("x2full", [npad, D], bf16, addr_space="Shared")

    with TileContext(nc) as tc:
        with (
            tc.tile_pool(name="const", bufs=1) as const,
            tc.tile_pool(name="prep", bufs=8) as prep,
            tc.tile_pool(name="msgs", bufs=2) as msgs,
            tc.tile_pool(name="spool", bufs=3) as spool,
            tc.tile_pool(name="epi", bufs=4) as epi,
            tc.tile_pool(name="pa", bufs=2, space="PSUM") as pa,
            tc.tile_pool(name="pt", bufs=2, space="PSUM") as pt,
            tc.tile_pool(name="pz", bufs=2, space="PSUM") as pz,
            tc.tile_pool(name="py", bufs=2, space="PSUM") as py,
        ):
            # ---- constants -------------------------------------------------
            def load_const(param, shape, dtype, tag):
                t = const.tile(shape, dtype, tag=tag)
                nc.sync.dma_start(t[:], param[:])
                return t

            degt_sb = load_const(degt_in, [128, nt], f32, "degt")
            dego_sb = load_const(dego_in, [128, b], f32, "dego")
            w1_sb = load_const(w1_in, [D, D], f32, "w1")
            w2_sb = load_const(w2_in, [D, D], f32, "w2")
            b1_sb = load_const(b1_in, [D, 1], f32, "b1")
            b2_sb = load_const(b2_in, [D, 1], f32, "b2")
            iota_sb = load_const(iota_in, [128, 128], bf16, "iota")
            identb_sb = load_const(identb_in, [128, 128], bf16, "identb")
            identf_sb = load_const(identf_in, [128, 128], f32, "identf")
            idxlo_sb = load_const(idxlo_in, [128, g * l16], i16, "idxlo")
            idxhi_sb = load_const(idxhi_in, [128, g * l16], i16, "idxhi")
            drel_sb = load_const(drel_in, [128, b * 2 * k2], bf16, "drel")

            rec_t = const.tile([128, nt], f32, tag="rec_t")
            nc.vector.reciprocal(rec_t[:], degt_sb[:])
            dis_t = const.tile([128, nt], f32, tag="dis_t")
            nc.scalar.activation(dis_t[:], rec_t[:], AF.Sqrt)
            rec_o = const.tile([128, b], f32, tag="rec_o")
            nc.vector.reciprocal(rec_o[:], dego_sb[:])
            dis_o = const.tile([128, b], f32, tag="dis_o")
            nc.scalar.activation(dis_o[:], rec_o[:], AF.Sqrt)

            w1b = const.tile([D, D], bf16, tag="w1b")
            nc.vector.tensor_copy(w1b[:], w1_sb[:])
            w2b = const.tile([D, D], bf16, tag="w2b")
            nc.vector.tensor_copy(w2b[:], w2_sb[:])

            if stage == "consts":
                dbg = prep.tile([128, b], f32, tag="dbgc")
                nc.vector.tensor_copy(dbg[:], dis_o[:])
                nc.sync.dma_start(out[0:128, 0:b], dbg[:])
            # ---- prep: x1 = bf16(x * dis) ---------------------------------
            for t in range(nt if stage == "consts" else 0, nt):
                xt = prep.tile([128, D], f32, tag="xt")
                nc.sync.dma_start(xt[:], x_in[t * 128 : (t + 1) * 128, :])
                xb = prep.tile([128, D], bf16, tag="xb")
                nc.vector.tensor_scalar(
                    xb[:], xt[:], dis_t[:, t : t + 1], None, OP.mult
                )
                nc.sync.dma_start(x1[t * 128 : (t + 1) * 128, :], xb[:])

            # ---- one GCN layer --------------------------------------------
            def layer(src_dram, first):
                srcs = (src_dram[0:half, :], src_dram[half:npad, :])
                idxs = (idxlo_sb, idxhi_sb)
                wb = w1b if first else w2b
                for gg in range(g):
                    msg = msgs.tile([128, 2 * gs * k2, D], bf16, tag="msg")
                    for h in (0, 1):
                        nc.gpsimd.dma_gather(
                            msg[:, h * gs * k2 : (h + 1) * gs * k2, :],
                            srcs[h],
                            idxs[h][:, gg * l16 : (gg + 1) * l16],
                            L,
                            L,
                            D,
                            single_packet=False,
                        )
                    for j in range(gs):
                        bb = gg * gs + j
                        S = spool.tile([128, 2 * k2, 128], bf16, tag="S")
                        nc.vector.tensor_tensor(
                            S[:, :, :],
                            iota_sb[:, :]
                            .rearrange("p (a c) -> p a c", a=1)
                            .broadcast_to([128, 2 * k2, 128]),
                            drel_sb[:, bb * 2 * k2 : (bb + 1) * 2 * k2]
                            .rearrange("p (a c) -> p a c", c=1)
                            .broadcast_to([128, 2 * k2, 128]),
                            OP.is_equal,
                        )
                        agg = pa.tile([128, D], f32, tag="agg")
                        for k in range(2 * k2):
                            ci = j * k2 + k if k < k2 else gs * k2 + j * k2 + (k - k2)
                            nc.tensor.matmul(
                                agg[:],
                                S[:, k, :],
                                msg[:, ci, :],
                                start=(k == 0),
                                stop=(k == 2 * k2 - 1),
                            )
                        # epilogue: dis[d]*agg -> ^T -> @W -> +b (relu) -> ^T
                        aggs = epi.tile([128, D], bf16, tag="aggs")
                        nc.scalar.activation(
                            aggs[:], agg[:], AF.Copy, scale=dis_o[:, bb : bb + 1]
                        )
                        aggT_p = pt.tile([128, D], bf16, tag="aggT_p")
                        nc.tensor.transpose(aggT_p[:], aggs[:], identb_sb[:])
                        aggT = epi.tile([128, D], bf16, tag="aggT")
                        nc.vector.tensor_copy(aggT[:], aggT_p[:])
                        z_p = pz.tile([128, D], f32, tag="z_p")
                        nc.tensor.matmul(
                            z_p[:], wb[:], aggT[:], start=True, stop=True
                        )
                        if first:
                            zs = epi.tile([128, D], bf16, tag="zs")
                            nc.scalar.activation(
                                zs[:], z_p[:], AF.Relu, bias=b1_sb[:, 0:1]
                            )
                            y_p = py.tile([128, D], bf16, tag="y_p")
                            nc.tensor.transpose(y_p[:], zs[:], identb_sb[:])
                            ys = epi.tile([128, D], bf16, tag="ys")
                            nc.vector.tensor_scalar(
                                ys[:], y_p[:], dis_o[:, bb : bb + 1], None, OP.mult
                            )
                            nc.sync.dma_start(
                                x2own[bb * 128 : (bb + 1) * 128, :], ys[:]
                            )
                        else:
                            zs = epi.tile([128, D], f32, tag="zs")
                            nc.scalar.activation(
                                zs[:], z_p[:], AF.Identity, bias=b2_sb[:, 0:1]
                            )
                            y_p = py.tile([128, D], f32, tag="y_p")
                            nc.tensor.transpose(y_p[:], zs[:], identf_sb[:])
                            ys = epi.tile([128, D], f32, tag="ys")
                            nc.vector.tensor_copy(ys[:], y_p[:])
                            nc.sync.dma_start(
                                out[bb * 128 : (bb + 1) * 128, :], ys[:]
                            )

            if stage == "consts":
                pass
            elif stage == "prep":
                # debug: copy x1 slice back out
                for t in range(b):
                    tt = prep.tile([128, D], bf16, tag="dbg")
                    nc.sync.dma_start(tt[:], x1[t * 128 : (t + 1) * 128, :])
                    o = prep.tile([128, D], f32, tag="dbgo")
                    nc.vector.tensor_copy(o[:], tt[:])
                    nc.sync.dma_start(out[t * 128 : (t + 1) * 128, :], o[:])
            elif stage == "l1":
                layer(x1, False)
            elif stage == "l1ag":
                layer(x1, True)
                nc.gpsimd.collective_compute(
                    "AllGather",
                    mybir.AluOpType.bypass,
                    replica_groups=[list(range(P))],
                    ins=[x2own[:]],
                    outs=[x2full[:]],
                )
                for t in range(b):
                    tt = prep.tile([128, D], bf16, tag="dbg")
                    nc.sync.dma_start(tt[:], x2full[t * 128 : (t + 1) * 128, :])
                    o = prep.tile([128, D], f32, tag="dbgo")
                    nc.vector.tensor_copy(o[:], tt[:])
                    nc.sync.dma_start(out[t * 128 : (t + 1) * 128, :], o[:])
            else:
                layer(x1, True)
                nc.gpsimd.collective_compute(
                    "AllGather",
                    mybir.AluOpType.bypass,
                    replica_groups=[list(range(P))],
                    ins=[x2own[:]],
                    outs=[x2full[:]],
                )
                layer(x2full, False)

    nc.finalize()
    return nc


def make_in_maps(pl, x, w1, b1, w2, b2):
    rpc, npad, b, half, nt, gs, g, k2, L, l16 = pl["sizes"]
    n = x.shape[0]
    x_pad = np.zeros((npad, D), dtype=np.float32)
    x_pad[:n] = x
    shared = {
        "x": x_pad,
        "deg_t": pl["deg_t"],
        "W1": np.ascontiguousarray(w1.astype(np.float32)),
        "b1": np.ascontiguousarray(b1.astype(np.float32).reshape(D, 1)),
        "W2": np.ascontiguousarray(w2.astype(np.float32)),
        "b2": np.ascontiguousarray(b2.astype(np.float32).reshape(D, 1)),
        "iota": pl["iota"],
        "ident_bf": pl["ident_bf"],
        "ident_f32": pl["ident_f32"],
    }
    in_maps = []
    for c in range(P):
        m = dict(shared)
        pc = pl["per_core"]
        m["deg_own"] = pc[c]["deg_own"]
        m["idx_lo"] = pc[c]["idx_lo"]
        m["idx_hi"] = pc[c]["idx_hi"]
        m["drel"] = pc[c]["drel"]
        in_maps.append(m)
    return in_maps


_CACHE = {}


def kernel(x, edge_index, W1, b1, W2, b2):
    from concourse.bass_utils import run_bass_kernel_spmd

    x = np.asarray(x)
    edge_index = np.asarray(edge_index)
    n = x.shape[0]
    pl = plan(edge_index, n)
    key = pl["sizes"]
    if key not in _CACHE:
        _CACHE[key] = build_program(pl)
    nc = _CACHE[key]
    in_maps = make_in_maps(pl, x, np.asarray(W1), np.asarray(b1), np.asarray(W2), np.asarray(b2))
    r = run_bass_kernel_spmd(nc, in_maps, list(range(P)))
    outs = [r.results[c]["out"] for c in range(P)]
    return np.concatenate(outs, axis=0)[:n].astype(np.float32)


# revision 15
# speedup vs baseline: 1.3314x; 1.3314x over previous
"""GCN encoder (2-layer, PyG GCNConv w/ self-loops + symmetric norm) on 8 trn2 cores.

Math per layer: out = dis * ((A+I)(dis*x)) @ W + b, with dis = deg^-1/2.
  x' = dis * x                         (row scale, bf16)
  agg[d] = sum_{edges s->d} x'[s]      (dma_gather + PE segment-sum vs one-hot S)
  y[d] = f(dis[d] * agg[d] @ W + b)    (f = relu for layer 1)
Sharding: destination nodes row-sharded across 8 cores (49 blocks of 128 each);
x' replicated via redundant prep (layer 1) and two pipelined AllGathers
(layer 2, A=28-block half / B=21-block half so gathers overlap the 2nd AG).
"""

import sys

sys.path.insert(0, "/opt/trn_rl_repo")

import numpy as np
import ml_dtypes

BF16 = ml_dtypes.bfloat16

D = 128
P = 8
BA = 28  # blocks per core in the "A" half (must be a multiple of group size)
GS = 7  # dest blocks per gather group


def _sizes(n):
    rpc = -(-n // (P * 128)) * 128  # rows per core, multiple of 128
    npad = rpc * P
    b = rpc // 128  # dest blocks per core
    nt = npad // 128
    gs = 1
    for d_ in range(1, 9):
        if b % d_ == 0:
            gs = d_
    g = b // gs
    ba = ((g + 1) // 2) * gs if g >= 2 else b  # A-half blocks, group-aligned
    ra, rb = ba * 128, (b - ba) * 128
    return rpc, npad, b, nt, gs, ba, ra, rb


def plan(edge_index, n):
    """Host-side integer preprocessing: degrees, per-core padded edge slots,
    gather index tiles, relative-destination tiles."""
    rpc, npad, b, nt, gs, ba, ra, rb = _sizes(n)
    src = edge_index[0].astype(np.int64)
    dst = edge_index[1].astype(np.int64)
    loops = np.arange(n, dtype=np.int64)
    allsrc = np.concatenate([src, loops])
    alldst = np.concatenate([dst, loops])

    deg = np.bincount(alldst, minlength=n).astype(np.float32)
    deg_pad = np.ones(npad, dtype=np.float32)
    deg_pad[:n] = deg
    deg_t = np.ascontiguousarray(deg_pad.reshape(nt, 128).T)  # [128, nt]

    core = alldst // rpc
    dloc = alldst - core * rpc
    blk = dloc >> 7
    drel = (dloc & 127).astype(np.float32)

    # source row mapping into the A/B halves
    s_core = allsrc // rpc
    s_w = allsrc - s_core * rpc
    hi = (s_w >= ra).astype(np.int64)
    s_idx = np.where(hi == 0, s_core * ra + s_w, s_core * rb + (s_w - ra))
    assert s_idx.max() < 32768

    key = (core * b + blk) * 2 + hi
    nkeys = P * b * 2
    counts = np.bincount(key, minlength=nkeys)
    k2 = max(1, int(-(-counts.max() // 128)))
    slots_per = k2 * 128

    order = np.argsort(key, kind="stable")
    key_s = key[order]
    run_start = np.zeros(nkeys, dtype=np.int64)
    np.cumsum(counts[:-1], out=run_start[1:])
    rank = np.arange(key_s.size, dtype=np.int64) - run_start[key_s]
    pos = key_s * slots_per + rank

    idx_slots = np.zeros(nkeys * slots_per, dtype=np.int16)
    drel_slots = np.full(nkeys * slots_per, -1.0, dtype=np.float32)
    idx_slots[pos] = s_idx[order].astype(np.int16)
    drel_slots[pos] = drel[order]

    idx_slots = idx_slots.reshape(P, b, 2, slots_per)
    drel_slots = drel_slots.reshape(P, b, 2, slots_per)

    g = b // gs
    L = gs * k2 * 128  # idxs per gather call
    l16 = L // 16

    def make_idx(core_slots, h):
        seq = core_slots[:, h, :].reshape(g, L)  # [g, L] block-major
        tile = seq.reshape(g, l16, 16).transpose(0, 2, 1)  # [g, 16, l16]
        tile = np.tile(tile, (1, 8, 1))  # [g, 128, l16]
        return np.ascontiguousarray(tile.transpose(1, 0, 2).reshape(128, g * l16))

    per_core = []
    for c in range(P):
        idx_lo = make_idx(idx_slots[c], 0)
        idx_hi = make_idx(idx_slots[c], 1)
        dr = drel_slots[c].reshape(b, 2 * k2, 128).transpose(2, 0, 1)
        dr = np.ascontiguousarray(dr.reshape(128, b * 2 * k2)).astype(BF16)
        deg_own = np.ascontiguousarray(deg_t[:, c * b : (c + 1) * b])
        per_core.append(
            {"idx_lo": idx_lo, "idx_hi": idx_hi, "drel": dr, "deg_own": deg_own}
        )

    # iota_rep[p, j*2k2 + c] = j  (chunk-minor layout for 2x-mode is_equal)
    iota_rep = np.repeat(np.arange(128, dtype=np.float32), 2 * k2)
    iota_rep = np.tile(iota_rep, (128, 1)).astype(BF16)
    ident = np.eye(128, dtype=np.float32)
    return {
        "sizes": (rpc, npad, b, nt, gs, ba, ra, rb, g, k2, L, l16),
        "deg_t": deg_t,
        "per_core": per_core,
        "iota_rep": iota_rep,
        "ident_bf": ident.astype(BF16),
        "ident_f32": ident,
    }


def build_program(pl):
    import concourse.mybir as mybir
    from concourse.bacc import Bacc
    from concourse.tile import TileContext

    rpc, npad, b, nt, gs, ba, ra, rb, g, k2, L, l16 = pl["sizes"]
    na, nb = P * ra, P * rb
    ga = ba // gs  # groups in the A half
    f32 = mybir.dt.float32
    bf16 = mybir.dt.bfloat16
    i16 = mybir.dt.int16
    AF = mybir.ActivationFunctionType
    OP = mybir.AluOpType

    nc = Bacc(num_devices=P)

    x_in = nc.declare_dram_parameter("x", [npad, D], f32, isOutput=False)
    degt_in = nc.declare_dram_parameter("deg_t", [128, nt], f32, isOutput=False)
    dego_in = nc.declare_dram_parameter("deg_own", [128, b], f32, isOutput=False)
    w1_in = nc.declare_dram_parameter("W1", [D, D], f32, isOutput=False)
    b1_in = nc.declare_dram_parameter("b1", [D, 1], f32, isOutput=False)
    w2_in = nc.declare_dram_parameter("W2", [D, D], f32, isOutput=False)
    b2_in = nc.declare_dram_parameter("b2", [D, 1], f32, isOutput=False)
    iota_in = nc.declare_dram_parameter("iota_rep", [128, 128 * 2 * k2], bf16, isOutput=False)
    identb_in = nc.declare_dram_parameter("ident_bf", [128, 128], bf16, isOutput=False)
    identf_in = nc.declare_dram_parameter("ident_f32", [128, 128], f32, isOutput=False)
    idxlo_in = nc.declare_dram_parameter("idx_lo", [128, g * l16], i16, isOutput=False)
    idxhi_in = nc.declare_dram_parameter("idx_hi", [128, g * l16], i16, isOutput=False)
    drel_in = nc.declare_dram_parameter("drel", [128, b * 2 * k2], bf16, isOutput=False)
    out = nc.declare_dram_parameter("out", [rpc, D], f32, isOutput=True)

    split = rb > 0
    x1a = nc.dram_tensor("x1a", [na, D], bf16)
    x2own_a = nc.dram_tensor("x2own_a", [ra, D], bf16)
    x2lo = nc.dram_tensor("x2lo", [na, D], bf16, addr_space="Shared")
    if split:
        x1b = nc.dram_tensor("x1b", [nb, D], bf16)
        x2own_b = nc.dram_tensor("x2own_b", [rb, D], bf16)
        x2hi = nc.dram_tensor("x2hi", [nb, D], bf16, addr_space="Shared")
    else:
        x1b, x2own_b, x2hi = x1a, None, x2lo

    with TileContext(nc) as tc:
        with (
            tc.tile_pool(name="const", bufs=1) as const,
            tc.tile_pool(name="prep", bufs=4) as prep,
            tc.tile_pool(name="msgs", bufs=2) as msgs,
            tc.tile_pool(name="spool", bufs=3) as spool,
            tc.tile_pool(name="epi", bufs=4) as epi,
            tc.tile_pool(name="pa", bufs=2, space="PSUM") as pa,
            tc.tile_pool(name="pt", bufs=2, space="PSUM") as pt,
            tc.tile_pool(name="pz", bufs=2, space="PSUM") as pz,
            tc.tile_pool(name="py", bufs=2, space="PSUM") as py,
        ):
            # ---- constants -------------------------------------------------
            def load_const(param, shape, dtype, tag):
                t = const.tile(shape, dtype, tag=tag)
                nc.sync.dma_start(t[:], param[:])
                return t

            degt_sb = load_const(degt_in, [128, nt], f32, "degt")
            dego_sb = load_const(dego_in, [128, b], f32, "dego")
            w1_sb = load_const(w1_in, [D, D], f32, "w1")
            w2_sb = load_const(w2_in, [D, D], f32, "w2")
            b1_sb = load_const(b1_in, [D, 1], f32, "b1")
            b2_sb = load_const(b2_in, [D, 1], f32, "b2")
            iota_sb = load_const(iota_in, [128, 128 * 2 * k2], bf16, "iota")
            identb_sb = load_const(identb_in, [128, 128], bf16, "identb")
            identf_sb = load_const(identf_in, [128, 128], f32, "identf")
            idxlo_sb = load_const(idxlo_in, [128, g * l16], i16, "idxlo")
            idxhi_sb = load_const(idxhi_in, [128, g * l16], i16, "idxhi")
            drel_sb = load_const(drel_in, [128, b * 2 * k2], bf16, "drel")

            rec_t = const.tile([128, nt], f32, tag="rec_t")
            nc.vector.reciprocal(rec_t[:], degt_sb[:])
            dis_t = const.tile([128, nt], f32, tag="dis_t")
            nc.scalar.activation(dis_t[:], rec_t[:], AF.Sqrt)
            rec_o = const.tile([128, b], f32, tag="rec_o")
            nc.vector.reciprocal(rec_o[:], dego_sb[:])
            dis_o = const.tile([128, b], f32, tag="dis_o")
            nc.scalar.activation(dis_o[:], rec_o[:], AF.Sqrt)

            w1b = const.tile([D, D], bf16, tag="w1b")
            nc.vector.tensor_copy(w1b[:], w1_sb[:])
            w2b = const.tile([D, D], bf16, tag="w2b")
            nc.vector.tensor_copy(w2b[:], w2_sb[:])

            # ---- prep: x1{a,b} = bf16(x * dis), batched --------------------
            def prep_run(tile0, ntiles, dstt, drow0):
                # process `ntiles` consecutive 128-row tiles starting at
                # global tile `tile0`, writing to dstt rows starting drow0
                off = 0
                while off < ntiles:
                    ch = min(14, ntiles - off)
                    t0 = tile0 + off
                    xt = prep.tile([128, 14, D], f32, tag="xt")
                    nc.sync.dma_start(
                        xt[:, 0:ch, :],
                        x_in[t0 * 128 : (t0 + ch) * 128, :].rearrange(
                            "(a p) d -> p a d", p=128
                        ),
                    )
                    xb = prep.tile([128, 14, D], bf16, tag="xb")
                    nc.vector.tensor_tensor(
                        xb[:, 0:ch, :],
                        xt[:, 0:ch, :],
                        dis_t[:, t0 : t0 + ch]
                        .rearrange("p (a c) -> p a c", c=1)
                        .broadcast_to([128, ch, D]),
                        OP.mult,
                    )
                    r0 = drow0 + off * 128
                    nc.sync.dma_start(
                        dstt[r0 : r0 + ch * 128, :].rearrange("(a p) d -> p a d", p=128),
                        xb[:, 0:ch, :],
                    )
                    off += ch

            for sc in range(P):
                prep_run(sc * b, ba, x1a, sc * ra)
                if split:
                    prep_run(sc * b + ba, b - ba, x1b, sc * rb)

            # ---- one GCN layer --------------------------------------------
            def do_group(gg, srcs, first):
                wb = w1b if first else w2b
                msg = msgs.tile([128, 2 * gs * k2, D], bf16, tag="msg")
                idxs = (idxlo_sb, idxhi_sb)
                for h in (0, 1):
                    nc.gpsimd.dma_gather(
                        msg[:, h * gs * k2 : (h + 1) * gs * k2, :],
                        srcs[h],
                        idxs[h][:, gg * l16 : (gg + 1) * l16],
                        L,
                        L,
                        D,
                        single_packet=False,
                    )
                for j in range(gs):
                    bb = gg * gs + j
                    S = spool.tile([128, 128, 2 * k2], bf16, tag="S")
                    nc.vector.tensor_tensor(
                        S[:, :, :],
                        iota_sb[:, :].rearrange("p (j c) -> p j c", j=128),
                        drel_sb[:, bb * 2 * k2 : (bb + 1) * 2 * k2]
                        .rearrange("p (a c) -> p a c", a=1)
                        .broadcast_to([128, 128, 2 * k2]),
                        OP.is_equal,
                    )
                    agg = pa.tile([128, D], f32, tag="agg")
                    for k in range(2 * k2):
                        ci = j * k2 + k if k < k2 else gs * k2 + j * k2 + (k - k2)
                        nc.tensor.matmul(
                            agg[:],
                            S[:, :, k],
                            msg[:, ci, :],
                            start=(k == 0),
                            stop=(k == 2 * k2 - 1),
                        )
                    # epilogue: dis[d]*agg -> ^T -> @W -> +b (relu) -> ^T
                    aggs = epi.tile([128, D], bf16, tag="aggs")
                    nc.scalar.activation(
                        aggs[:], agg[:], AF.Copy, scale=dis_o[:, bb : bb + 1]
                    )
                    aggT_p = pt.tile([128, D], bf16, tag="aggT_p")
                    nc.tensor.transpose(aggT_p[:], aggs[:], identb_sb[:])
                    aggT = epi.tile([128, D], bf16, tag="aggT")
                    nc.vector.tensor_copy(aggT[:], aggT_p[:])
                    z_p = pz.tile([128, D], f32, tag="z_p")
                    nc.tensor.matmul(z_p[:], wb[:], aggT[:], start=True, stop=True)
                    if first:
                        zs = epi.tile([128, D], bf16, tag="zs")
                        nc.scalar.activation(
                            zs[:], z_p[:], AF.Relu, bias=b1_sb[:, 0:1]
                        )
                        y_p = py.tile([128, D], bf16, tag="y_p")
                        nc.tensor.transpose(y_p[:], zs[:], identb_sb[:])
                        ys = epi.tile([128, D], bf16, tag="ys")
                        nc.vector.tensor_scalar(
                            ys[:], y_p[:], dis_o[:, bb : bb + 1], None, OP.mult
                        )
                        if bb < ba:
                            nc.sync.dma_start(
                                x2own_a[bb * 128 : (bb + 1) * 128, :], ys[:]
                            )
                        else:
                            r0 = (bb - ba) * 128
                            nc.sync.dma_start(x2own_b[r0 : r0 + 128, :], ys[:])
                    else:
                        zs = epi.tile([128, D], f32, tag="zs2")
                        nc.scalar.activation(
                            zs[:], z_p[:], AF.Identity, bias=b2_sb[:, 0:1]
                        )
                        y_p = py.tile([128, D], f32, tag="y_p")
                        nc.tensor.transpose(y_p[:], zs[:], identf_sb[:])
                        ys = epi.tile([128, D], f32, tag="ys2")
                        nc.vector.tensor_copy(ys[:], y_p[:])
                        nc.sync.dma_start(out[bb * 128 : (bb + 1) * 128, :], ys[:])

            # layer 1 (A groups, AG1, B groups, AG2)
            for gg in range(ga):
                do_group(gg, (x1a[:, :], x1b[:, :]), True)
            nc.gpsimd.collective_compute(
                "AllGather",
                mybir.AluOpType.bypass,
                replica_groups=[list(range(P))],
                ins=[x2own_a[:]],
                outs=[x2lo[:]],
            )
            for gg in range(ga, g):
                do_group(gg, (x1a[:, :], x1b[:, :]), True)
            if split:
                nc.gpsimd.collective_compute(
                    "AllGather",
                    mybir.AluOpType.bypass,
                    replica_groups=[list(range(P))],
                    ins=[x2own_b[:]],
                    outs=[x2hi[:]],
                )
            # layer 2
            for gg in range(g):
                do_group(gg, (x2lo[:, :], x2hi[:, :]), False)

    nc.finalize()
    return nc


def make_in_maps(pl, x, w1, b1, w2, b2):
    rpc, npad, b, nt, gs, ba, ra, rb, g, k2, L, l16 = pl["sizes"]
    n = x.shape[0]
    x_pad = np.zeros((npad, D), dtype=np.float32)
    x_pad[:n] = x
    shared = {
        "x": x_pad,
        "deg_t": pl["deg_t"],
        "W1": np.ascontiguousarray(w1.astype(np.float32)),
        "b1": np.ascontiguousarray(b1.astype(np.float32).reshape(D, 1)),
        "W2": np.ascontiguousarray(w2.astype(np.float32)),
        "b2": np.ascontiguousarray(b2.astype(np.float32).reshape(D, 1)),
        "iota_rep": pl["iota_rep"],
        "ident_bf": pl["ident_bf"],
        "ident_f32": pl["ident_f32"],
    }
    in_maps = []
    for c in range(P):
        m = dict(shared)
        pc = pl["per_core"]
        m["deg_own"] = pc[c]["deg_own"]
        m["idx_lo"] = pc[c]["idx_lo"]
        m["idx_hi"] = pc[c]["idx_hi"]
        m["drel"] = pc[c]["drel"]
        in_maps.append(m)
    return in_maps


_CACHE = {}


def kernel(x, edge_index, W1, b1, W2, b2):
    from concourse.bass_utils import run_bass_kernel_spmd

    x = np.asarray(x)
    edge_index = np.asarray(edge_index)
    n = x.shape[0]
    pl = plan(edge_index, n)
    key = pl["sizes"]
    if key not in _CACHE:
        _CACHE[key] = build_program(pl)
    nc = _CACHE[key]
    in_maps = make_in_maps(
        pl, x, np.asarray(W1), np.asarray(b1), np.asarray(W2), np.asarray(b2)
    )
    r = run_bass_kernel_spmd(nc, in_maps, list(range(P)))
    outs = [r.results[c]["out"] for c in range(P)]
    return np.concatenate(outs, axis=0)[:n].astype(np.float32)


# revision 17
# speedup vs baseline: 1.3837x; 1.0393x over previous
"""GCN encoder (2-layer, PyG GCNConv w/ self-loops + symmetric norm) on 8 trn2 cores.

Math per layer: out = dis * ((A+I)(dis*x)) @ W + b, with dis = deg^-1/2.
  x' = dis * x                         (row scale, bf16)
  agg[d] = sum_{edges s->d} x'[s]      (dma_gather + PE segment-sum vs one-hot S)
  y[d] = f(dis[d] * agg[d] @ W + b)    (f = relu for layer 1)
Sharding: destination nodes row-sharded across 8 cores (49 blocks of 128 each);
x' replicated via redundant prep (layer 1) and two pipelined AllGathers
(layer 2, A=28-block half / B=21-block half so gathers overlap the 2nd AG).
"""

import sys

sys.path.insert(0, "/opt/trn_rl_repo")

import numpy as np
import ml_dtypes

BF16 = ml_dtypes.bfloat16

D = 128
P = 8
BA = 28  # blocks per core in the "A" half (must be a multiple of group size)
GS = 7  # dest blocks per gather group


def _sizes(n):
    rpc = -(-n // (P * 128)) * 128  # rows per core, multiple of 128
    npad = rpc * P
    b = rpc // 128  # dest blocks per core
    nt = npad // 128
    gs = 1
    for d_ in range(1, 9):
        if b % d_ == 0:
            gs = d_
    g = b // gs
    ba = ((g + 1) // 2) * gs if g >= 2 else b  # A-half blocks, group-aligned
    ra, rb = ba * 128, (b - ba) * 128
    return rpc, npad, b, nt, gs, ba, ra, rb


def plan(edge_index, n):
    """Host-side integer preprocessing: degrees, per-core padded edge slots,
    gather index tiles, relative-destination tiles."""
    rpc, npad, b, nt, gs, ba, ra, rb = _sizes(n)
    src = edge_index[0].astype(np.int64)
    dst = edge_index[1].astype(np.int64)
    loops = np.arange(n, dtype=np.int64)
    allsrc = np.concatenate([src, loops])
    alldst = np.concatenate([dst, loops])

    deg = np.bincount(alldst, minlength=n).astype(np.float32)
    deg_pad = np.ones(npad, dtype=np.float32)
    deg_pad[:n] = deg
    deg_t = np.ascontiguousarray(deg_pad.reshape(nt, 128).T)  # [128, nt]

    core = alldst // rpc
    dloc = alldst - core * rpc
    blk = dloc >> 7
    drel = (dloc & 127).astype(np.float32)

    # source row mapping into the A/B halves
    s_core = allsrc // rpc
    s_w = allsrc - s_core * rpc
    hi = (s_w >= ra).astype(np.int64)
    s_idx = np.where(hi == 0, s_core * ra + s_w, s_core * rb + (s_w - ra))
    assert s_idx.max() < 32768

    key = (core * b + blk) * 2 + hi
    nkeys = P * b * 2
    counts = np.bincount(key, minlength=nkeys)
    cc = counts.reshape(P, b, 2)
    k2lo = max(1, int(-(-cc[:, :, 0].max() // 128)))
    k2hi = max(1, int(-(-cc[:, :, 1].max() // 128)))
    spl, sph = k2lo * 128, k2hi * 128

    order = np.argsort(key, kind="stable")
    key_s = key[order]
    run_start = np.zeros(nkeys, dtype=np.int64)
    np.cumsum(counts[:-1], out=run_start[1:])
    rank = np.arange(key_s.size, dtype=np.int64) - run_start[key_s]
    # per-key slot base: lo keys (even) get spl slots, hi keys (odd) sph
    slot_base = np.zeros(nkeys + 1, dtype=np.int64)
    per_key = np.where(np.arange(nkeys) % 2 == 0, spl, sph)
    np.cumsum(per_key, out=slot_base[1:])
    pos = slot_base[key_s] + rank

    tot = int(slot_base[-1])
    idx_flat = np.zeros(tot, dtype=np.int16)
    drel_flat = np.full(tot, -1.0, dtype=np.float32)
    idx_flat[pos] = s_idx[order].astype(np.int16)
    drel_flat[pos] = drel[order]

    # -> [P, b, spl+sph] then split
    per_blk = idx_flat.reshape(P, b, spl + sph)
    idx_lo_s = per_blk[:, :, :spl]
    idx_hi_s = per_blk[:, :, spl:]
    drel_blk = drel_flat.reshape(P, b, spl + sph)

    g = b // gs
    k2t = k2lo + k2hi
    L_lo = gs * spl
    L_hi = gs * sph
    l16lo, l16hi = L_lo // 16, L_hi // 16

    def make_idx(core_slots, L):
        l16 = L // 16
        seq = core_slots.reshape(g, L)  # [g, L] block-major
        tile = seq.reshape(g, l16, 16).transpose(0, 2, 1)  # [g, 16, l16]
        tile = np.tile(tile, (1, 8, 1))  # [g, 128, l16]
        return np.ascontiguousarray(tile.transpose(1, 0, 2).reshape(128, g * l16))

    per_core = []
    for c in range(P):
        idx_lo = make_idx(idx_lo_s[c], L_lo)
        idx_hi = make_idx(idx_hi_s[c], L_hi)
        # drel device layout [128, b*k2t], chunk-major per block, lo then hi
        dr = drel_blk[c].reshape(b, k2t, 128).transpose(2, 0, 1)
        dr = np.ascontiguousarray(dr.reshape(128, b * k2t)).astype(BF16)
        deg_own = np.ascontiguousarray(deg_t[:, c * b : (c + 1) * b])
        per_core.append(
            {"idx_lo": idx_lo, "idx_hi": idx_hi, "drel": dr, "deg_own": deg_own}
        )

    # iota_rep[p, j*k2t + c] = j  (chunk-minor layout for 2x-mode is_equal)
    iota_rep = np.repeat(np.arange(128, dtype=np.float32), k2t)
    iota_rep = np.tile(iota_rep, (128, 1)).astype(BF16)
    ident = np.eye(128, dtype=np.float32)
    return {
        "sizes": (rpc, npad, b, nt, gs, ba, ra, rb, g, k2lo, k2hi, l16lo, l16hi),
        "deg_t": deg_t,
        "per_core": per_core,
        "iota_rep": iota_rep,
        "ident_bf": ident.astype(BF16),
        "ident_f32": ident,
    }


def build_program(pl):
    import concourse.mybir as mybir
    from concourse.bacc import Bacc
    from concourse.tile import TileContext

    rpc, npad, b, nt, gs, ba, ra, rb, g, k2lo, k2hi, l16lo, l16hi = pl["sizes"]
    na, nb = P * ra, P * rb
    ga = ba // gs  # groups in the A half
    k2t = k2lo + k2hi
    L_lo, L_hi = 16 * l16lo, 16 * l16hi
    f32 = mybir.dt.float32
    bf16 = mybir.dt.bfloat16
    i16 = mybir.dt.int16
    AF = mybir.ActivationFunctionType
    OP = mybir.AluOpType

    nc = Bacc(num_devices=P)

    x_in = nc.declare_dram_parameter("x", [npad, D], f32, isOutput=False)
    degt_in = nc.declare_dram_parameter("deg_t", [128, nt], f32, isOutput=False)
    dego_in = nc.declare_dram_parameter("deg_own", [128, b], f32, isOutput=False)
    w1_in = nc.declare_dram_parameter("W1", [D, D], f32, isOutput=False)
    b1_in = nc.declare_dram_parameter("b1", [D, 1], f32, isOutput=False)
    w2_in = nc.declare_dram_parameter("W2", [D, D], f32, isOutput=False)
    b2_in = nc.declare_dram_parameter("b2", [D, 1], f32, isOutput=False)
    iota_in = nc.declare_dram_parameter("iota_rep", [128, 128 * k2t], bf16, isOutput=False)
    identb_in = nc.declare_dram_parameter("ident_bf", [128, 128], bf16, isOutput=False)
    identf_in = nc.declare_dram_parameter("ident_f32", [128, 128], f32, isOutput=False)
    idxlo_in = nc.declare_dram_parameter("idx_lo", [128, g * l16lo], i16, isOutput=False)
    idxhi_in = nc.declare_dram_parameter("idx_hi", [128, g * l16hi], i16, isOutput=False)
    drel_in = nc.declare_dram_parameter("drel", [128, b * k2t], bf16, isOutput=False)
    out = nc.declare_dram_parameter("out", [rpc, D], f32, isOutput=True)

    split = rb > 0
    x1a = nc.dram_tensor("x1a", [na, D], bf16)
    x2own_a = nc.dram_tensor("x2own_a", [ra, D], bf16)
    x2lo = nc.dram_tensor("x2lo", [na, D], bf16, addr_space="Shared")
    if split:
        x1b = nc.dram_tensor("x1b", [nb, D], bf16)
        x2own_b = nc.dram_tensor("x2own_b", [rb, D], bf16)
        x2hi = nc.dram_tensor("x2hi", [nb, D], bf16, addr_space="Shared")
    else:
        x1b, x2own_b, x2hi = x1a, None, x2lo

    with TileContext(nc) as tc:
        with (
            tc.tile_pool(name="const", bufs=1) as const,
            tc.tile_pool(name="prep", bufs=4) as prep,
            tc.tile_pool(name="msgs", bufs=2) as msgs,
            tc.tile_pool(name="spool", bufs=3) as spool,
            tc.tile_pool(name="epi", bufs=4) as epi,
            tc.tile_pool(name="pa", bufs=2, space="PSUM") as pa,
            tc.tile_pool(name="pt", bufs=2, space="PSUM") as pt,
            tc.tile_pool(name="pz", bufs=2, space="PSUM") as pz,
            tc.tile_pool(name="py", bufs=2, space="PSUM") as py,
        ):
            # ---- constants -------------------------------------------------
            def load_const(param, shape, dtype, tag):
                t = const.tile(shape, dtype, tag=tag)
                nc.sync.dma_start(t[:], param[:])
                return t

            degt_sb = load_const(degt_in, [128, nt], f32, "degt")
            dego_sb = load_const(dego_in, [128, b], f32, "dego")
            w1_sb = load_const(w1_in, [D, D], f32, "w1")
            w2_sb = load_const(w2_in, [D, D], f32, "w2")
            b1_sb = load_const(b1_in, [D, 1], f32, "b1")
            b2_sb = load_const(b2_in, [D, 1], f32, "b2")
            iota_sb = load_const(iota_in, [128, 128 * k2t], bf16, "iota")
            identb_sb = load_const(identb_in, [128, 128], bf16, "identb")
            identf_sb = load_const(identf_in, [128, 128], f32, "identf")
            idxlo_sb = load_const(idxlo_in, [128, g * l16lo], i16, "idxlo")
            idxhi_sb = load_const(idxhi_in, [128, g * l16hi], i16, "idxhi")
            drel_sb = load_const(drel_in, [128, b * k2t], bf16, "drel")

            rec_t = const.tile([128, nt], f32, tag="rec_t")
            nc.vector.reciprocal(rec_t[:], degt_sb[:])
            dis_t = const.tile([128, nt], f32, tag="dis_t")
            nc.scalar.activation(dis_t[:], rec_t[:], AF.Sqrt)
            rec_o = const.tile([128, b], f32, tag="rec_o")
            nc.vector.reciprocal(rec_o[:], dego_sb[:])
            dis_o = const.tile([128, b], f32, tag="dis_o")
            nc.scalar.activation(dis_o[:], rec_o[:], AF.Sqrt)

            w1b = const.tile([D, D], bf16, tag="w1b")
            nc.vector.tensor_copy(w1b[:], w1_sb[:])
            w2b = const.tile([D, D], bf16, tag="w2b")
            nc.vector.tensor_copy(w2b[:], w2_sb[:])

            # ---- prep: x1{a,b} = bf16(x * dis), batched --------------------
            def prep_run(tile0, ntiles, dstt, drow0):
                # process `ntiles` consecutive 128-row tiles starting at
                # global tile `tile0`, writing to dstt rows starting drow0
                off = 0
                while off < ntiles:
                    ch = min(14, ntiles - off)
                    t0 = tile0 + off
                    xt = prep.tile([128, 14, D], f32, tag="xt")
                    nc.sync.dma_start(
                        xt[:, 0:ch, :],
                        x_in[t0 * 128 : (t0 + ch) * 128, :].rearrange(
                            "(a p) d -> p a d", p=128
                        ),
                    )
                    xb = prep.tile([128, 14, D], bf16, tag="xb")
                    nc.vector.tensor_tensor(
                        xb[:, 0:ch, :],
                        xt[:, 0:ch, :],
                        dis_t[:, t0 : t0 + ch]
                        .rearrange("p (a c) -> p a c", c=1)
                        .broadcast_to([128, ch, D]),
                        OP.mult,
                    )
                    r0 = drow0 + off * 128
                    nc.sync.dma_start(
                        dstt[r0 : r0 + ch * 128, :].rearrange("(a p) d -> p a d", p=128),
                        xb[:, 0:ch, :],
                    )
                    off += ch

            for sc in range(P):
                prep_run(sc * b, ba, x1a, sc * ra)
            if split:
                for sc in range(P):
                    prep_run(sc * b + ba, b - ba, x1b, sc * rb)

            # ---- one GCN layer --------------------------------------------
            def do_group(gg, srcs, first):
                wb = w1b if first else w2b
                msg = msgs.tile([128, gs * k2t, D], bf16, tag="msg")
                nc.gpsimd.dma_gather(
                    msg[:, 0 : gs * k2lo, :],
                    srcs[0],
                    idxlo_sb[:, gg * l16lo : (gg + 1) * l16lo],
                    L_lo,
                    L_lo,
                    D,
                    single_packet=False,
                )
                nc.gpsimd.dma_gather(
                    msg[:, gs * k2lo : gs * k2t, :],
                    srcs[1],
                    idxhi_sb[:, gg * l16hi : (gg + 1) * l16hi],
                    L_hi,
                    L_hi,
                    D,
                    single_packet=False,
                )
                for j in range(gs):
                    bb = gg * gs + j
                    S = spool.tile([128, 128, k2t], bf16, tag="S")
                    nc.vector.tensor_tensor(
                        S[:, :, :],
                        iota_sb[:, :].rearrange("p (j c) -> p j c", j=128),
                        drel_sb[:, bb * k2t : (bb + 1) * k2t]
                        .rearrange("p (a c) -> p a c", a=1)
                        .broadcast_to([128, 128, k2t]),
                        OP.is_equal,
                    )
                    agg = pa.tile([128, D], f32, tag="agg")
                    for k in range(k2t):
                        ci = (
                            j * k2lo + k
                            if k < k2lo
                            else gs * k2lo + j * k2hi + (k - k2lo)
                        )
                        nc.tensor.matmul(
                            agg[:],
                            S[:, :, k],
                            msg[:, ci, :],
                            start=(k == 0),
                            stop=(k == k2t - 1),
                        )
                    # epilogue: dis[d]*agg -> ^T -> @W -> +b (relu) -> ^T
                    aggs = epi.tile([128, D], bf16, tag="aggs")
                    nc.scalar.activation(
                        aggs[:], agg[:], AF.Copy, scale=dis_o[:, bb : bb + 1]
                    )
                    aggT_p = pt.tile([128, D], bf16, tag="aggT_p")
                    nc.tensor.transpose(aggT_p[:], aggs[:], identb_sb[:])
                    aggT = epi.tile([128, D], bf16, tag="aggT")
                    nc.vector.tensor_copy(aggT[:], aggT_p[:])
                    z_p = pz.tile([128, D], f32, tag="z_p")
                    nc.tensor.matmul(z_p[:], wb[:], aggT[:], start=True, stop=True)
                    if first:
                        zs = epi.tile([128, D], bf16, tag="zs")
                        nc.scalar.activation(
                            zs[:], z_p[:], AF.Relu, bias=b1_sb[:, 0:1]
                        )
                        y_p = py.tile([128, D], bf16, tag="y_p")
                        nc.tensor.transpose(y_p[:], zs[:], identb_sb[:])
                        ys = epi.tile([128, D], bf16, tag="ys")
                        nc.vector.tensor_scalar(
                            ys[:], y_p[:], dis_o[:, bb : bb + 1], None, OP.mult
                        )
                        if bb < ba:
                            nc.sync.dma_start(
                                x2own_a[bb * 128 : (bb + 1) * 128, :], ys[:]
                            )
                        else:
                            r0 = (bb - ba) * 128
                            nc.sync.dma_start(x2own_b[r0 : r0 + 128, :], ys[:])
                    else:
                        zs = epi.tile([128, D], f32, tag="zs2")
                        nc.scalar.activation(
                            zs[:], z_p[:], AF.Identity, bias=b2_sb[:, 0:1]
                        )
                        y_p = py.tile([128, D], f32, tag="y_p")
                        nc.tensor.transpose(y_p[:], zs[:], identf_sb[:])
                        ys = epi.tile([128, D], f32, tag="ys2")
                        nc.vector.tensor_copy(ys[:], y_p[:])
                        nc.sync.dma_start(out[bb * 128 : (bb + 1) * 128, :], ys[:])

            # layer 1 (A groups, AG1, B groups, AG2)
            for gg in range(ga):
                do_group(gg, (x1a[:, :], x1b[:, :]), True)
            nc.gpsimd.collective_compute(
                "AllGather",
                mybir.AluOpType.bypass,
                replica_groups=[list(range(P))],
                ins=[x2own_a[:]],
                outs=[x2lo[:]],
            )
            for gg in range(ga, g):
                do_group(gg, (x1a[:, :], x1b[:, :]), True)
            if split:
                nc.gpsimd.collective_compute(
                    "AllGather",
                    mybir.AluOpType.bypass,
                    replica_groups=[list(range(P))],
                    ins=[x2own_b[:]],
                    outs=[x2hi[:]],
                )
            # layer 2
            for gg in range(g):
                do_group(gg, (x2lo[:, :], x2hi[:, :]), False)

    nc.finalize()
    return nc


def make_in_maps(pl, x, w1, b1, w2, b2):
    rpc, npad, b, nt, gs, ba, ra, rb, g, k2lo, k2hi, l16lo, l16hi = pl["sizes"]
    n = x.shape[0]
    x_pad = np.zeros((npad, D), dtype=np.float32)
    x_pad[:n] = x
    shared = {
        "x": x_pad,
        "deg_t": pl["deg_t"],
        "W1": np.ascontiguousarray(w1.astype(np.float32)),
        "b1": np.ascontiguousarray(b1.astype(np.float32).reshape(D, 1)),
        "W2": np.ascontiguousarray(w2.astype(np.float32)),
        "b2": np.ascontiguousarray(b2.astype(np.float32).reshape(D, 1)),
        "iota_rep": pl["iota_rep"],
        "ident_bf": pl["ident_bf"],
        "ident_f32": pl["ident_f32"],
    }
    in_maps = []
    for c in range(P):
        m = dict(shared)
        pc = pl["per_core"]
        m["deg_own"] = pc[c]["deg_own"]
        m["idx_lo"] = pc[c]["idx_lo"]
        m["idx_hi"] = pc[c]["idx_hi"]
        m["drel"] = pc[c]["drel"]
        in_maps.append(m)
    return in_maps


_CACHE = {}


def kernel(x, edge_index, W1, b1, W2, b2):
    from concourse.bass_utils import run_bass_kernel_spmd

    x = np.asarray(x)
    edge_index = np.asarray(edge_index)
    n = x.shape[0]
    pl = plan(edge_index, n)
    key = pl["sizes"]
    if key not in _CACHE:
        _CACHE[key] = build_program(pl)
    nc = _CACHE[key]
    in_maps = make_in_maps(
        pl, x, np.asarray(W1), np.asarray(b1), np.asarray(W2), np.asarray(b2)
    )
    r = run_bass_kernel_spmd(nc, in_maps, list(range(P)))
    outs = [r.results[c]["out"] for c in range(P)]
    return np.concatenate(outs, axis=0)[:n].astype(np.float32)


# revision 18
# speedup vs baseline: 1.5134x; 1.0937x over previous
"""GCN encoder (2-layer, PyG GCNConv w/ self-loops + symmetric norm) on 8 trn2 cores.

Math per layer: out = dis * ((A+I)(dis*x)) @ W + b, with dis = deg^-1/2.
  x' = dis * x                         (row scale, bf16)
  agg[d] = sum_{edges s->d} x'[s]      (dma_gather + PE segment-sum vs one-hot S)
  y[d] = f(dis[d] * agg[d] @ W + b)    (f = relu for layer 1)
Sharding: destination nodes row-sharded across 8 cores (49 blocks of 128 each);
x' replicated via redundant prep (layer 1) and two pipelined AllGathers
(layer 2, A=28-block half / B=21-block half so gathers overlap the 2nd AG).
"""

import sys

sys.path.insert(0, "/opt/trn_rl_repo")

import numpy as np
import ml_dtypes

BF16 = ml_dtypes.bfloat16

D = 128
P = 8
BA = 28  # blocks per core in the "A" half (must be a multiple of group size)
GS = 7  # dest blocks per gather group


def _sizes(n):
    rpc = -(-n // (P * 128)) * 128  # rows per core, multiple of 128
    npad = rpc * P
    b = rpc // 128  # dest blocks per core
    nt = npad // 128
    gs = 1
    for d_ in range(1, 9):
        if b % d_ == 0:
            gs = d_
    g = b // gs
    ba = ((g + 1) // 2) * gs if g >= 2 else b  # A-half blocks, group-aligned
    ra, rb = ba * 128, (b - ba) * 128
    return rpc, npad, b, nt, gs, ba, ra, rb


def plan(edge_index, n):
    """Host-side integer preprocessing: degrees, per-core padded edge slots,
    gather index tiles, relative-destination tiles."""
    rpc, npad, b, nt, gs, ba, ra, rb = _sizes(n)
    src = edge_index[0].astype(np.int64)
    dst = edge_index[1].astype(np.int64)
    loops = np.arange(n, dtype=np.int64)
    allsrc = np.concatenate([src, loops])
    alldst = np.concatenate([dst, loops])

    deg = np.bincount(alldst, minlength=n).astype(np.float32)
    deg_pad = np.ones(npad, dtype=np.float32)
    deg_pad[:n] = deg
    deg_t = np.ascontiguousarray(deg_pad.reshape(nt, 128).T)  # [128, nt]

    core = alldst // rpc
    dloc = alldst - core * rpc
    blk = dloc >> 7
    drel = (dloc & 127).astype(np.float32)

    # source row mapping into the A/B halves
    s_core = allsrc // rpc
    s_w = allsrc - s_core * rpc
    hi = (s_w >= ra).astype(np.int64)
    s_idx = np.where(hi == 0, s_core * ra + s_w, s_core * rb + (s_w - ra))
    assert s_idx.max() < 32768

    key = (core * b + blk) * 2 + hi
    nkeys = P * b * 2
    counts = np.bincount(key, minlength=nkeys)
    cc = counts.reshape(P, b, 2)
    k2lo = max(1, int(-(-cc[:, :, 0].max() // 128)))
    k2hi = max(1, int(-(-cc[:, :, 1].max() // 128)))
    spl, sph = k2lo * 128, k2hi * 128

    order = np.argsort(key, kind="stable")
    key_s = key[order]
    run_start = np.zeros(nkeys, dtype=np.int64)
    np.cumsum(counts[:-1], out=run_start[1:])
    rank = np.arange(key_s.size, dtype=np.int64) - run_start[key_s]
    # per-key slot base: lo keys (even) get spl slots, hi keys (odd) sph
    slot_base = np.zeros(nkeys + 1, dtype=np.int64)
    per_key = np.where(np.arange(nkeys) % 2 == 0, spl, sph)
    np.cumsum(per_key, out=slot_base[1:])
    pos = slot_base[key_s] + rank

    tot = int(slot_base[-1])
    idx_flat = np.zeros(tot, dtype=np.int16)
    drel_flat = np.full(tot, -1.0, dtype=np.float32)
    idx_flat[pos] = s_idx[order].astype(np.int16)
    drel_flat[pos] = drel[order]

    # -> [P, b, spl+sph] then split
    per_blk = idx_flat.reshape(P, b, spl + sph)
    idx_lo_s = per_blk[:, :, :spl]
    idx_hi_s = per_blk[:, :, spl:]
    drel_blk = drel_flat.reshape(P, b, spl + sph)

    g = b // gs
    k2t = k2lo + k2hi
    L_lo = gs * spl
    L_hi = gs * sph
    l16lo, l16hi = L_lo // 16, L_hi // 16

    def make_idx(core_slots, L):
        l16 = L // 16
        seq = core_slots.reshape(g, L)  # [g, L] block-major
        tile = seq.reshape(g, l16, 16).transpose(0, 2, 1)  # [g, 16, l16]
        tile = np.tile(tile, (1, 8, 1))  # [g, 128, l16]
        return np.ascontiguousarray(tile.transpose(1, 0, 2).reshape(128, g * l16))

    per_core = []
    for c in range(P):
        idx_lo = make_idx(idx_lo_s[c], L_lo)
        idx_hi = make_idx(idx_hi_s[c], L_hi)
        # drel device layout [128, b*k2t], chunk-major per block, lo then hi
        dr = drel_blk[c].reshape(b, k2t, 128).transpose(2, 0, 1)
        dr = np.ascontiguousarray(dr.reshape(128, b * k2t)).astype(BF16)
        deg_own = np.ascontiguousarray(deg_t[:, c * b : (c + 1) * b])
        per_core.append(
            {"idx_lo": idx_lo, "idx_hi": idx_hi, "drel": dr, "deg_own": deg_own}
        )

    # iota_rep[p, j*k2t + c] = j  (chunk-minor layout for 2x-mode is_equal)
    iota_rep = np.repeat(np.arange(128, dtype=np.float32), k2t)
    iota_rep = np.tile(iota_rep, (128, 1)).astype(BF16)
    ident = np.eye(128, dtype=np.float32)
    return {
        "sizes": (rpc, npad, b, nt, gs, ba, ra, rb, g, k2lo, k2hi, l16lo, l16hi),
        "deg_t": deg_t,
        "per_core": per_core,
        "iota_rep": iota_rep,
        "ident_bf": ident.astype(BF16),
        "ident_f32": ident,
    }


def build_program(pl):
    import concourse.mybir as mybir
    from concourse.bacc import Bacc
    from concourse.tile import TileContext

    rpc, npad, b, nt, gs, ba, ra, rb, g, k2lo, k2hi, l16lo, l16hi = pl["sizes"]
    na, nb = P * ra, P * rb
    ga = ba // gs  # groups in the A half
    k2t = k2lo + k2hi
    L_lo, L_hi = 16 * l16lo, 16 * l16hi
    f32 = mybir.dt.float32
    bf16 = mybir.dt.bfloat16
    i16 = mybir.dt.int16
    AF = mybir.ActivationFunctionType
    OP = mybir.AluOpType

    nc = Bacc(num_devices=P)

    x_in = nc.declare_dram_parameter("x", [npad, D], f32, isOutput=False)
    degt_in = nc.declare_dram_parameter("deg_t", [128, nt], f32, isOutput=False)
    dego_in = nc.declare_dram_parameter("deg_own", [128, b], f32, isOutput=False)
    w1_in = nc.declare_dram_parameter("W1", [D, D], f32, isOutput=False)
    b1_in = nc.declare_dram_parameter("b1", [D, 1], f32, isOutput=False)
    w2_in = nc.declare_dram_parameter("W2", [D, D], f32, isOutput=False)
    b2_in = nc.declare_dram_parameter("b2", [D, 1], f32, isOutput=False)
    iota_in = nc.declare_dram_parameter("iota_rep", [128, 128 * k2t], bf16, isOutput=False)
    identb_in = nc.declare_dram_parameter("ident_bf", [128, 128], bf16, isOutput=False)
    identf_in = nc.declare_dram_parameter("ident_f32", [128, 128], f32, isOutput=False)
    idxlo_in = nc.declare_dram_parameter("idx_lo", [128, g * l16lo], i16, isOutput=False)
    idxhi_in = nc.declare_dram_parameter("idx_hi", [128, g * l16hi], i16, isOutput=False)
    drel_in = nc.declare_dram_parameter("drel", [128, b * k2t], bf16, isOutput=False)
    out = nc.declare_dram_parameter("out", [rpc, D], f32, isOutput=True)

    split = rb > 0
    x1a = nc.dram_tensor("x1a", [na, D], bf16)
    x2own_a = nc.dram_tensor("x2own_a", [ra, D], bf16)
    x2lo = nc.dram_tensor("x2lo", [na, D], bf16, addr_space="Shared")
    if split:
        x1b = nc.dram_tensor("x1b", [nb, D], bf16)
        x2own_b = nc.dram_tensor("x2own_b", [rb, D], bf16)
        x2hi = nc.dram_tensor("x2hi", [nb, D], bf16, addr_space="Shared")
    else:
        x1b, x2own_b, x2hi = x1a, None, x2lo

    with TileContext(nc) as tc:
        with (
            tc.tile_pool(name="const", bufs=1) as const,
            tc.tile_pool(name="prep", bufs=4) as prep,
            tc.tile_pool(name="msgs", bufs=2) as msgs,
            tc.tile_pool(name="spool", bufs=3) as spool,
            tc.tile_pool(name="epi", bufs=4) as epi,
            tc.tile_pool(name="pa", bufs=2, space="PSUM") as pa,
            tc.tile_pool(name="pt", bufs=2, space="PSUM") as pt,
            tc.tile_pool(name="pz", bufs=2, space="PSUM") as pz,
            tc.tile_pool(name="py", bufs=2, space="PSUM") as py,
        ):
            # ---- constants -------------------------------------------------
            def load_const(param, shape, dtype, tag):
                t = const.tile(shape, dtype, tag=tag)
                nc.sync.dma_start(t[:], param[:])
                return t

            degt_sb = load_const(degt_in, [128, nt], f32, "degt")
            dego_sb = load_const(dego_in, [128, b], f32, "dego")
            w1_sb = load_const(w1_in, [D, D], f32, "w1")
            w2_sb = load_const(w2_in, [D, D], f32, "w2")
            b1_sb = load_const(b1_in, [D, 1], f32, "b1")
            b2_sb = load_const(b2_in, [D, 1], f32, "b2")
            iota_sb = load_const(iota_in, [128, 128 * k2t], bf16, "iota")
            identb_sb = load_const(identb_in, [128, 128], bf16, "identb")
            identf_sb = load_const(identf_in, [128, 128], f32, "identf")
            idxlo_sb = load_const(idxlo_in, [128, g * l16lo], i16, "idxlo")
            idxhi_sb = load_const(idxhi_in, [128, g * l16hi], i16, "idxhi")
            drel_sb = load_const(drel_in, [128, b * k2t], bf16, "drel")

            rec_t = const.tile([128, nt], f32, tag="rec_t")
            nc.vector.reciprocal(rec_t[:], degt_sb[:])
            dis_t = const.tile([128, nt], f32, tag="dis_t")
            nc.scalar.activation(dis_t[:], rec_t[:], AF.Sqrt)
            rec_o = const.tile([128, b], f32, tag="rec_o")
            nc.vector.reciprocal(rec_o[:], dego_sb[:])
            dis_o = const.tile([128, b], f32, tag="dis_o")
            nc.scalar.activation(dis_o[:], rec_o[:], AF.Sqrt)

            w1b = const.tile([D, D], bf16, tag="w1b")
            nc.vector.tensor_copy(w1b[:], w1_sb[:])
            w2b = const.tile([D, D], bf16, tag="w2b")
            nc.vector.tensor_copy(w2b[:], w2_sb[:])

            # ---- prep: x1{a,b} = bf16(x * dis), batched --------------------
            def prep_run(tile0, ntiles, dstt, drow0):
                # process `ntiles` consecutive 128-row tiles starting at
                # global tile `tile0`, writing to dstt rows starting drow0
                off = 0
                while off < ntiles:
                    ch = min(14, ntiles - off)
                    t0 = tile0 + off
                    xt = prep.tile([128, 14, D], f32, tag="xt")
                    nc.sync.dma_start(
                        xt[:, 0:ch, :],
                        x_in[t0 * 128 : (t0 + ch) * 128, :].rearrange(
                            "(a p) d -> p a d", p=128
                        ),
                    )
                    xb = prep.tile([128, 14, D], bf16, tag="xb")
                    nc.vector.tensor_tensor(
                        xb[:, 0:ch, :],
                        xt[:, 0:ch, :],
                        dis_t[:, t0 : t0 + ch]
                        .rearrange("p (a c) -> p a c", c=1)
                        .broadcast_to([128, ch, D]),
                        OP.mult,
                    )
                    r0 = drow0 + off * 128
                    nc.sync.dma_start(
                        dstt[r0 : r0 + ch * 128, :].rearrange("(a p) d -> p a d", p=128),
                        xb[:, 0:ch, :],
                    )
                    off += ch

            for sc in range(P):
                prep_run(sc * b, ba, x1a, sc * ra)
            if split:
                for sc in range(P):
                    prep_run(sc * b + ba, b - ba, x1b, sc * rb)

            # ---- one GCN layer, split into lo/hi phases -------------------
            # partial[:, bb*128:(bb+1)*128] holds dis[d] * sum(lo msgs), f32
            partial = const.tile([128, b * 128], f32, tag="partial")

            def do_lo(gg, src_lo):
                msg = msgs.tile([128, gs * k2lo, D], bf16, tag="msg")
                nc.gpsimd.dma_gather(
                    msg[:, :, :],
                    src_lo,
                    idxlo_sb[:, gg * l16lo : (gg + 1) * l16lo],
                    L_lo,
                    L_lo,
                    D,
                    single_packet=False,
                )
                for j in range(gs):
                    bb = gg * gs + j
                    S = spool.tile([128, 128, k2t], bf16, tag="S")
                    nc.vector.tensor_tensor(
                        S[:, :, 0:k2lo],
                        iota_sb[:, :]
                        .rearrange("p (j c) -> p j c", j=128)[:, :, 0:k2lo],
                        drel_sb[:, bb * k2t : bb * k2t + k2lo]
                        .rearrange("p (a c) -> p a c", a=1)
                        .broadcast_to([128, 128, k2lo]),
                        OP.is_equal,
                    )
                    agg = pa.tile([128, D], f32, tag="agg")
                    for k in range(k2lo):
                        nc.tensor.matmul(
                            agg[:],
                            S[:, :, k],
                            msg[:, j * k2lo + k, :],
                            start=(k == 0),
                            stop=(k == k2lo - 1),
                        )
                    nc.scalar.activation(
                        partial[:, bb * 128 : (bb + 1) * 128],
                        agg[:],
                        AF.Copy,
                        scale=dis_o[:, bb : bb + 1],
                    )

            def do_hi(gg, src_hi, first):
                wb = w1b if first else w2b
                msg = msgs.tile([128, gs * k2hi, D], bf16, tag="msg")
                nc.gpsimd.dma_gather(
                    msg[:, :, :],
                    src_hi,
                    idxhi_sb[:, gg * l16hi : (gg + 1) * l16hi],
                    L_hi,
                    L_hi,
                    D,
                    single_packet=False,
                )
                for j in range(gs):
                    bb = gg * gs + j
                    S = spool.tile([128, 128, k2t], bf16, tag="S")
                    nc.vector.tensor_tensor(
                        S[:, :, 0:k2hi],
                        iota_sb[:, :]
                        .rearrange("p (j c) -> p j c", j=128)[:, :, 0:k2hi],
                        drel_sb[:, bb * k2t + k2lo : (bb + 1) * k2t]
                        .rearrange("p (a c) -> p a c", a=1)
                        .broadcast_to([128, 128, k2hi]),
                        OP.is_equal,
                    )
                    agg = pa.tile([128, D], f32, tag="agg")
                    for k in range(k2hi):
                        nc.tensor.matmul(
                            agg[:],
                            S[:, :, k],
                            msg[:, j * k2hi + k, :],
                            start=(k == 0),
                            stop=(k == k2hi - 1),
                        )
                    # aggs = dis[d]*agg_hi + partial  (bf16)
                    aggs = epi.tile([128, D], bf16, tag="aggs")
                    nc.vector.scalar_tensor_tensor(
                        aggs[:],
                        agg[:],
                        dis_o[:, bb : bb + 1],
                        partial[:, bb * 128 : (bb + 1) * 128],
                        OP.mult,
                        OP.add,
                    )
                    aggT_p = pt.tile([128, D], bf16, tag="aggT_p")
                    nc.tensor.transpose(aggT_p[:], aggs[:], identb_sb[:])
                    aggT = epi.tile([128, D], bf16, tag="aggT")
                    nc.vector.tensor_copy(aggT[:], aggT_p[:])
                    z_p = pz.tile([128, D], f32, tag="z_p")
                    nc.tensor.matmul(z_p[:], wb[:], aggT[:], start=True, stop=True)
                    if first:
                        zs = epi.tile([128, D], bf16, tag="zs")
                        nc.scalar.activation(
                            zs[:], z_p[:], AF.Relu, bias=b1_sb[:, 0:1]
                        )
                        y_p = py.tile([128, D], bf16, tag="y_p")
                        nc.tensor.transpose(y_p[:], zs[:], identb_sb[:])
                        ys = epi.tile([128, D], bf16, tag="ys")
                        nc.vector.tensor_scalar(
                            ys[:], y_p[:], dis_o[:, bb : bb + 1], None, OP.mult
                        )
                        if bb < ba:
                            nc.sync.dma_start(
                                x2own_a[bb * 128 : (bb + 1) * 128, :], ys[:]
                            )
                        else:
                            r0 = (bb - ba) * 128
                            nc.sync.dma_start(x2own_b[r0 : r0 + 128, :], ys[:])
                    else:
                        zs = epi.tile([128, D], f32, tag="zs2")
                        nc.scalar.activation(
                            zs[:], z_p[:], AF.Identity, bias=b2_sb[:, 0:1]
                        )
                        y_p = py.tile([128, D], f32, tag="y_p")
                        nc.tensor.transpose(y_p[:], zs[:], identf_sb[:])
                        ys = epi.tile([128, D], f32, tag="ys2")
                        nc.vector.tensor_copy(ys[:], y_p[:])
                        nc.sync.dma_start(out[bb * 128 : (bb + 1) * 128, :], ys[:])

            # layer 1: A-half groups (lo+hi), AG1, B-half groups, AG2
            for gg in range(ga):
                do_lo(gg, x1a[:, :])
            for gg in range(ga):
                do_hi(gg, x1b[:, :], True)
            nc.gpsimd.collective_compute(
                "AllGather",
                mybir.AluOpType.bypass,
                replica_groups=[list(range(P))],
                ins=[x2own_a[:]],
                outs=[x2lo[:]],
            )
            for gg in range(ga, g):
                do_lo(gg, x1a[:, :])
            for gg in range(ga, g):
                do_hi(gg, x1b[:, :], True)
            if split:
                nc.gpsimd.collective_compute(
                    "AllGather",
                    mybir.AluOpType.bypass,
                    replica_groups=[list(range(P))],
                    ins=[x2own_b[:]],
                    outs=[x2hi[:]],
                )
            # layer 2: all lo phases (need only AG1), then hi phases (AG2)
            for gg in range(g):
                do_lo(gg, x2lo[:, :])
            for gg in range(g):
                do_hi(gg, x2hi[:, :], False)

    nc.finalize()
    return nc


def make_in_maps(pl, x, w1, b1, w2, b2):
    rpc, npad, b, nt, gs, ba, ra, rb, g, k2lo, k2hi, l16lo, l16hi = pl["sizes"]
    n = x.shape[0]
    x_pad = np.zeros((npad, D), dtype=np.float32)
    x_pad[:n] = x
    shared = {
        "x": x_pad,
        "deg_t": pl["deg_t"],
        "W1": np.ascontiguousarray(w1.astype(np.float32)),
        "b1": np.ascontiguousarray(b1.astype(np.float32).reshape(D, 1)),
        "W2": np.ascontiguousarray(w2.astype(np.float32)),
        "b2": np.ascontiguousarray(b2.astype(np.float32).reshape(D, 1)),
        "iota_rep": pl["iota_rep"],
        "ident_bf": pl["ident_bf"],
        "ident_f32": pl["ident_f32"],
    }
    in_maps = []
    for c in range(P):
        m = dict(shared)
        pc = pl["per_core"]
        m["deg_own"] = pc[c]["deg_own"]
        m["idx_lo"] = pc[c]["idx_lo"]
        m["idx_hi"] = pc[c]["idx_hi"]
        m["drel"] = pc[c]["drel"]
        in_maps.append(m)
    return in_maps


_CACHE = {}


def kernel(x, edge_index, W1, b1, W2, b2):
    from concourse.bass_utils import run_bass_kernel_spmd

    x = np.asarray(x)
    edge_index = np.asarray(edge_index)
    n = x.shape[0]
    pl = plan(edge_index, n)
    key = pl["sizes"]
    if key not in _CACHE:
        _CACHE[key] = build_program(pl)
    nc = _CACHE[key]
    in_maps = make_in_maps(
        pl, x, np.asarray(W1), np.asarray(b1), np.asarray(W2), np.asarray(b2)
    )
    r = run_bass_kernel_spmd(nc, in_maps, list(range(P)))
    outs = [r.results[c]["out"] for c in range(P)]
    return np.concatenate(outs, axis=0)[:n].astype(np.float32)


# revision 19
# speedup vs baseline: 1.5355x; 1.0146x over previous
"""GCN encoder (2-layer, PyG GCNConv w/ self-loops + symmetric norm) on 8 trn2 cores.

Math per layer: out = dis * ((A+I)(dis*x)) @ W + b, with dis = deg^-1/2.
  x' = dis * x                         (row scale, bf16)
  agg[d] = sum_{edges s->d} x'[s]      (dma_gather + PE segment-sum vs one-hot S)
  y[d] = f(dis[d] * agg[d] @ W + b)    (f = relu for layer 1)
Sharding: destination nodes row-sharded across 8 cores (49 blocks of 128 each);
x' replicated via redundant prep (layer 1) and two pipelined AllGathers
(layer 2, A=28-block half / B=21-block half so gathers overlap the 2nd AG).
"""

import sys

sys.path.insert(0, "/opt/trn_rl_repo")

import numpy as np
import ml_dtypes

BF16 = ml_dtypes.bfloat16

D = 128
P = 8
BA = 28  # blocks per core in the "A" half (must be a multiple of group size)
GS = 7  # dest blocks per gather group


def _sizes(n):
    rpc = -(-n // (P * 128)) * 128  # rows per core, multiple of 128
    npad = rpc * P
    b = rpc // 128  # dest blocks per core
    nt = npad // 128
    gs = 1
    for d_ in range(1, 9):
        if b % d_ == 0:
            gs = d_
    g = b // gs
    ba = ((g + 1) // 2) * gs if g >= 2 else b  # A-half blocks, group-aligned
    ra, rb = ba * 128, (b - ba) * 128
    return rpc, npad, b, nt, gs, ba, ra, rb


def plan(edge_index, n):
    """Host-side integer preprocessing: degrees, per-core padded edge slots,
    gather index tiles, relative-destination tiles."""
    rpc, npad, b, nt, gs, ba, ra, rb = _sizes(n)
    src = edge_index[0].astype(np.int64)
    dst = edge_index[1].astype(np.int64)
    loops = np.arange(n, dtype=np.int64)
    allsrc = np.concatenate([src, loops])
    alldst = np.concatenate([dst, loops])

    deg = np.bincount(alldst, minlength=n).astype(np.float32)
    deg_pad = np.ones(npad, dtype=np.float32)
    deg_pad[:n] = deg
    deg_t = np.ascontiguousarray(deg_pad.reshape(nt, 128).T)  # [128, nt]

    core = alldst // rpc
    dloc = alldst - core * rpc
    blk = dloc >> 7
    drel = (dloc & 127).astype(np.float32)

    # source row mapping into the A/B halves
    s_core = allsrc // rpc
    s_w = allsrc - s_core * rpc
    hi = (s_w >= ra).astype(np.int64)
    s_idx = np.where(hi == 0, s_core * ra + s_w, s_core * rb + (s_w - ra))
    assert s_idx.max() < 32768

    key = (core * b + blk) * 2 + hi
    nkeys = P * b * 2
    counts = np.bincount(key, minlength=nkeys)
    cc = counts.reshape(P, b, 2)
    k2lo = max(1, int(-(-cc[:, :, 0].max() // 128)))
    k2hi = max(1, int(-(-cc[:, :, 1].max() // 128)))
    spl, sph = k2lo * 128, k2hi * 128

    order = np.argsort(key, kind="stable")
    key_s = key[order]
    run_start = np.zeros(nkeys, dtype=np.int64)
    np.cumsum(counts[:-1], out=run_start[1:])
    rank = np.arange(key_s.size, dtype=np.int64) - run_start[key_s]
    # per-key slot base: lo keys (even) get spl slots, hi keys (odd) sph
    slot_base = np.zeros(nkeys + 1, dtype=np.int64)
    per_key = np.where(np.arange(nkeys) % 2 == 0, spl, sph)
    np.cumsum(per_key, out=slot_base[1:])
    pos = slot_base[key_s] + rank

    tot = int(slot_base[-1])
    idx_flat = np.zeros(tot, dtype=np.int16)
    drel_flat = np.full(tot, -1.0, dtype=np.float32)
    idx_flat[pos] = s_idx[order].astype(np.int16)
    drel_flat[pos] = drel[order]

    # -> [P, b, spl+sph] then split
    per_blk = idx_flat.reshape(P, b, spl + sph)
    idx_lo_s = per_blk[:, :, :spl]
    idx_hi_s = per_blk[:, :, spl:]
    drel_blk = drel_flat.reshape(P, b, spl + sph)

    g = b // gs
    k2t = k2lo + k2hi
    L_lo = gs * spl
    L_hi = gs * sph
    l16lo, l16hi = L_lo // 16, L_hi // 16

    def make_idx(core_slots, L):
        l16 = L // 16
        seq = core_slots.reshape(g, L)  # [g, L] block-major
        tile = seq.reshape(g, l16, 16).transpose(0, 2, 1)  # [g, 16, l16]
        tile = np.tile(tile, (1, 8, 1))  # [g, 128, l16]
        return np.ascontiguousarray(tile.transpose(1, 0, 2).reshape(128, g * l16))

    per_core = []
    for c in range(P):
        idx_lo = make_idx(idx_lo_s[c], L_lo)
        idx_hi = make_idx(idx_hi_s[c], L_hi)
        # drel device layout [128, b*k2t], chunk-major per block, lo then hi
        dr = drel_blk[c].reshape(b, k2t, 128).transpose(2, 0, 1)
        dr = np.ascontiguousarray(dr.reshape(128, b * k2t)).astype(BF16)
        deg_own = np.ascontiguousarray(deg_t[:, c * b : (c + 1) * b])
        per_core.append(
            {"idx_lo": idx_lo, "idx_hi": idx_hi, "drel": dr, "deg_own": deg_own}
        )

    # iota_rep[p, j*k2t + c] = j  (chunk-minor layout for 2x-mode is_equal)
    iota_rep = np.repeat(np.arange(128, dtype=np.float32), k2t)
    iota_rep = np.tile(iota_rep, (128, 1)).astype(BF16)
    ident = np.eye(128, dtype=np.float32)
    return {
        "sizes": (rpc, npad, b, nt, gs, ba, ra, rb, g, k2lo, k2hi, l16lo, l16hi),
        "deg_t": deg_t,
        "per_core": per_core,
        "iota_rep": iota_rep,
        "ident_bf": ident.astype(BF16),
        "ident_f32": ident,
    }


def build_program(pl):
    import concourse.mybir as mybir
    from concourse.bacc import Bacc
    from concourse.tile import TileContext

    rpc, npad, b, nt, gs, ba, ra, rb, g, k2lo, k2hi, l16lo, l16hi = pl["sizes"]
    na, nb = P * ra, P * rb
    ga = ba // gs  # groups in the A half
    k2t = k2lo + k2hi
    L_lo, L_hi = 16 * l16lo, 16 * l16hi
    f32 = mybir.dt.float32
    bf16 = mybir.dt.bfloat16
    i16 = mybir.dt.int16
    AF = mybir.ActivationFunctionType
    OP = mybir.AluOpType

    nc = Bacc(num_devices=P)

    x_in = nc.declare_dram_parameter("x", [npad, D], f32, isOutput=False)
    degt_in = nc.declare_dram_parameter("deg_t", [128, nt], f32, isOutput=False)
    dego_in = nc.declare_dram_parameter("deg_own", [128, b], f32, isOutput=False)
    w1_in = nc.declare_dram_parameter("W1", [D, D], f32, isOutput=False)
    b1_in = nc.declare_dram_parameter("b1", [D, 1], f32, isOutput=False)
    w2_in = nc.declare_dram_parameter("W2", [D, D], f32, isOutput=False)
    b2_in = nc.declare_dram_parameter("b2", [D, 1], f32, isOutput=False)
    b2t_in = nc.declare_dram_parameter("b2_tile", [D, D], f32, isOutput=False)
    iota_in = nc.declare_dram_parameter("iota_rep", [128, 128 * k2t], bf16, isOutput=False)
    identb_in = nc.declare_dram_parameter("ident_bf", [128, 128], bf16, isOutput=False)
    identf_in = nc.declare_dram_parameter("ident_f32", [128, 128], f32, isOutput=False)
    idxlo_in = nc.declare_dram_parameter("idx_lo", [128, g * l16lo], i16, isOutput=False)
    idxhi_in = nc.declare_dram_parameter("idx_hi", [128, g * l16hi], i16, isOutput=False)
    drel_in = nc.declare_dram_parameter("drel", [128, b * k2t], bf16, isOutput=False)
    out = nc.declare_dram_parameter("out", [rpc, D], f32, isOutput=True)

    split = rb > 0
    x1a = nc.dram_tensor("x1a", [na, D], bf16)
    x2own_a = nc.dram_tensor("x2own_a", [ra, D], bf16)
    x2lo = nc.dram_tensor("x2lo", [na, D], bf16, addr_space="Shared")
    if split:
        x1b = nc.dram_tensor("x1b", [nb, D], bf16)
        x2own_b = nc.dram_tensor("x2own_b", [rb, D], bf16)
        x2hi = nc.dram_tensor("x2hi", [nb, D], bf16, addr_space="Shared")
    else:
        x1b, x2own_b, x2hi = x1a, None, x2lo

    with TileContext(nc) as tc:
        with (
            tc.tile_pool(name="const", bufs=1) as const,
            tc.tile_pool(name="prep", bufs=4) as prep,
            tc.tile_pool(name="msgs", bufs=2) as msgs,
            tc.tile_pool(name="spool", bufs=3) as spool,
            tc.tile_pool(name="epi", bufs=4) as epi,
            tc.tile_pool(name="pa", bufs=2, space="PSUM") as pa,
            tc.tile_pool(name="pt", bufs=2, space="PSUM") as pt,
            tc.tile_pool(name="pz", bufs=2, space="PSUM") as pz,
            tc.tile_pool(name="py", bufs=2, space="PSUM") as py,
        ):
            # ---- constants -------------------------------------------------
            def load_const(param, shape, dtype, tag):
                t = const.tile(shape, dtype, tag=tag)
                nc.sync.dma_start(t[:], param[:])
                return t

            degt_sb = load_const(degt_in, [128, nt], f32, "degt")
            dego_sb = load_const(dego_in, [128, b], f32, "dego")
            w1_sb = load_const(w1_in, [D, D], f32, "w1")
            w2_sb = load_const(w2_in, [D, D], f32, "w2")
            b1_sb = load_const(b1_in, [D, 1], f32, "b1")
            b2_sb = load_const(b2_in, [D, 1], f32, "b2")
            b2t_sb = load_const(b2t_in, [D, D], f32, "b2t")
            iota_sb = load_const(iota_in, [128, 128 * k2t], bf16, "iota")
            identb_sb = load_const(identb_in, [128, 128], bf16, "identb")
            identf_sb = load_const(identf_in, [128, 128], f32, "identf")
            idxlo_sb = load_const(idxlo_in, [128, g * l16lo], i16, "idxlo")
            idxhi_sb = load_const(idxhi_in, [128, g * l16hi], i16, "idxhi")
            drel_sb = load_const(drel_in, [128, b * k2t], bf16, "drel")

            rec_t = const.tile([128, nt], f32, tag="rec_t")
            nc.vector.reciprocal(rec_t[:], degt_sb[:])
            dis_t = const.tile([128, nt], f32, tag="dis_t")
            nc.scalar.activation(dis_t[:], rec_t[:], AF.Sqrt)
            rec_o = const.tile([128, b], f32, tag="rec_o")
            nc.vector.reciprocal(rec_o[:], dego_sb[:])
            dis_o = const.tile([128, b], f32, tag="dis_o")
            nc.scalar.activation(dis_o[:], rec_o[:], AF.Sqrt)

            w1b = const.tile([D, D], bf16, tag="w1b")
            nc.vector.tensor_copy(w1b[:], w1_sb[:])
            w2b = const.tile([D, D], bf16, tag="w2b")
            nc.vector.tensor_copy(w2b[:], w2_sb[:])

            # ---- prep: x1{a,b} = bf16(x * dis), batched --------------------
            def prep_run(tile0, ntiles, dstt, drow0):
                # process `ntiles` consecutive 128-row tiles starting at
                # global tile `tile0`, writing to dstt rows starting drow0
                off = 0
                while off < ntiles:
                    ch = min(14, ntiles - off)
                    t0 = tile0 + off
                    xt = prep.tile([128, 14, D], f32, tag="xt")
                    nc.sync.dma_start(
                        xt[:, 0:ch, :],
                        x_in[t0 * 128 : (t0 + ch) * 128, :].rearrange(
                            "(a p) d -> p a d", p=128
                        ),
                    )
                    xb = prep.tile([128, 14, D], bf16, tag="xb")
                    for i in range(ch):
                        nc.scalar.activation(
                            xb[:, i, :],
                            xt[:, i, :],
                            AF.Copy,
                            scale=dis_t[:, t0 + i : t0 + i + 1],
                        )
                    r0 = drow0 + off * 128
                    nc.sync.dma_start(
                        dstt[r0 : r0 + ch * 128, :].rearrange("(a p) d -> p a d", p=128),
                        xb[:, 0:ch, :],
                    )
                    off += ch

            for sc in range(P):
                prep_run(sc * b, ba, x1a, sc * ra)
            if split:
                for sc in range(P):
                    prep_run(sc * b + ba, b - ba, x1b, sc * rb)

            # ---- one GCN layer, two source-half phases --------------------
            # phase A accumulates dis[d]*sum(msgs of one half) into partial;
            # phase B adds the other half, then runs the block epilogue.
            partial = const.tile([128, b * 128], f32, tag="partial")
            halves = {
                0: (idxlo_sb, L_lo, l16lo, k2lo, 0),
                1: (idxhi_sb, L_hi, l16hi, k2hi, k2lo),
            }

            def gather_half(gg, src, h):
                idx_sb, L, l16, k2h, _ = halves[h]
                msg = msgs.tile([128, gs * k2h, D], bf16, tag="msg")
                nc.gpsimd.dma_gather(
                    msg[:, :, :],
                    src,
                    idx_sb[:, gg * l16 : (gg + 1) * l16],
                    L,
                    L,
                    D,
                    single_packet=False,
                )
                return msg

            def block_agg(j, bb, msg, h):
                _, _, _, k2h, koff = halves[h]
                S = spool.tile([128, 128, k2t], bf16, tag="S")
                nc.vector.tensor_tensor(
                    S[:, :, 0:k2h],
                    iota_sb[:, :]
                    .rearrange("p (j c) -> p j c", j=128)[:, :, 0:k2h],
                    drel_sb[:, bb * k2t + koff : bb * k2t + koff + k2h]
                    .rearrange("p (a c) -> p a c", a=1)
                    .broadcast_to([128, 128, k2h]),
                    OP.is_equal,
                )
                agg = pa.tile([128, D], f32, tag="agg")
                for k in range(k2h):
                    nc.tensor.matmul(
                        agg[:],
                        S[:, :, k],
                        msg[:, j * k2h + k, :],
                        start=(k == 0),
                        stop=(k == k2h - 1),
                    )
                return agg

            def do_phase_a(gg, src, h):
                msg = gather_half(gg, src, h)
                for j in range(gs):
                    bb = gg * gs + j
                    agg = block_agg(j, bb, msg, h)
                    nc.scalar.activation(
                        partial[:, bb * 128 : (bb + 1) * 128],
                        agg[:],
                        AF.Copy,
                        scale=dis_o[:, bb : bb + 1],
                    )

            def do_phase_b(gg, src, h, first):
                wb = w1b if first else w2b
                msg = gather_half(gg, src, h)
                for j in range(gs):
                    bb = gg * gs + j
                    agg = block_agg(j, bb, msg, h)
                    # aggs = dis[d]*agg + partial  (bf16)
                    aggs = epi.tile([128, D], bf16, tag="aggs")
                    nc.vector.scalar_tensor_tensor(
                        aggs[:],
                        agg[:],
                        dis_o[:, bb : bb + 1],
                        partial[:, bb * 128 : (bb + 1) * 128],
                        OP.mult,
                        OP.add,
                    )
                    aggT_p = pt.tile([128, D], bf16, tag="aggT_p")
                    nc.tensor.transpose(aggT_p[:], aggs[:], identb_sb[:])
                    aggT = epi.tile([128, D], bf16, tag="aggT")
                    nc.scalar.activation(aggT[:], aggT_p[:], AF.Copy)
                    if first:
                        z_p = pz.tile([128, D], f32, tag="z_p")
                        nc.tensor.matmul(
                            z_p[:], wb[:], aggT[:], start=True, stop=True
                        )
                        zs = epi.tile([128, D], bf16, tag="zs")
                        nc.scalar.activation(
                            zs[:], z_p[:], AF.Relu, bias=b1_sb[:, 0:1]
                        )
                        y_p = py.tile([128, D], bf16, tag="y_p")
                        nc.tensor.transpose(y_p[:], zs[:], identb_sb[:])
                        ys = epi.tile([128, D], bf16, tag="ys")
                        nc.vector.tensor_scalar(
                            ys[:], y_p[:], dis_o[:, bb : bb + 1], None, OP.mult
                        )
                        if bb < ba:
                            nc.sync.dma_start(
                                x2own_a[bb * 128 : (bb + 1) * 128, :], ys[:]
                            )
                        else:
                            r0 = (bb - ba) * 128
                            nc.sync.dma_start(x2own_b[r0 : r0 + 128, :], ys[:])
                    else:
                        # direct [dest, dhid] = aggT.T @ W, then + b2 tile
                        z_p = pz.tile([128, D], f32, tag="z_p")
                        nc.tensor.matmul(
                            z_p[:], aggT[:], wb[:], start=True, stop=True
                        )
                        ys = epi.tile([128, D], f32, tag="ys2")
                        nc.vector.scalar_tensor_tensor(
                            ys[:], z_p[:], 1.0, b2t_sb[:], OP.mult, OP.add
                        )
                        nc.sync.dma_start(out[bb * 128 : (bb + 1) * 128, :], ys[:])

            # layer 1: B-half dest groups first so AG_B can start early
            for gg in range(ga, g):
                do_phase_a(gg, x1a[:, :], 0)
            for gg in range(ga, g):
                do_phase_b(gg, x1b[:, :], 1, True)
            if split:
                nc.gpsimd.collective_compute(
                    "AllGather",
                    mybir.AluOpType.bypass,
                    replica_groups=[list(range(P))],
                    ins=[x2own_b[:]],
                    outs=[x2hi[:]],
                )
            for gg in range(ga):
                do_phase_a(gg, x1a[:, :], 0)
            for gg in range(ga):
                do_phase_b(gg, x1b[:, :], 1, True)
            nc.gpsimd.collective_compute(
                "AllGather",
                mybir.AluOpType.bypass,
                replica_groups=[list(range(P))],
                ins=[x2own_a[:]],
                outs=[x2lo[:]],
            )
            # layer 2: hi phase first (needs only AG_B), lo merge phase after AG_A
            for gg in range(g):
                do_phase_a(gg, x2hi[:, :], 1)
            for gg in range(g):
                do_phase_b(gg, x2lo[:, :], 0, False)

    nc.finalize()
    return nc


def make_in_maps(pl, x, w1, b1, w2, b2):
    rpc, npad, b, nt, gs, ba, ra, rb, g, k2lo, k2hi, l16lo, l16hi = pl["sizes"]
    n = x.shape[0]
    x_pad = np.zeros((npad, D), dtype=np.float32)
    x_pad[:n] = x
    shared = {
        "x": x_pad,
        "deg_t": pl["deg_t"],
        "W1": np.ascontiguousarray(w1.astype(np.float32)),
        "b1": np.ascontiguousarray(b1.astype(np.float32).reshape(D, 1)),
        "W2": np.ascontiguousarray(w2.astype(np.float32)),
        "b2": np.ascontiguousarray(b2.astype(np.float32).reshape(D, 1)),
        "b2_tile": np.ascontiguousarray(
            np.tile(b2.astype(np.float32).reshape(1, D), (D, 1))
        ),
        "iota_rep": pl["iota_rep"],
        "ident_bf": pl["ident_bf"],
        "ident_f32": pl["ident_f32"],
    }
    in_maps = []
    for c in range(P):
        m = dict(shared)
        pc = pl["per_core"]
        m["deg_own"] = pc[c]["deg_own"]
        m["idx_lo"] = pc[c]["idx_lo"]
        m["idx_hi"] = pc[c]["idx_hi"]
        m["drel"] = pc[c]["drel"]
        in_maps.append(m)
    return in_maps


_CACHE = {}


def kernel(x, edge_index, W1, b1, W2, b2):
    from concourse.bass_utils import run_bass_kernel_spmd

    x = np.asarray(x)
    edge_index = np.asarray(edge_index)
    n = x.shape[0]
    pl = plan(edge_index, n)
    key = pl["sizes"]
    if key not in _CACHE:
        _CACHE[key] = build_program(pl)
    nc = _CACHE[key]
    in_maps = make_in_maps(
        pl, x, np.asarray(W1), np.asarray(b1), np.asarray(W2), np.asarray(b2)
    )
    r = run_bass_kernel_spmd(nc, in_maps, list(range(P)))
    outs = [r.results[c]["out"] for c in range(P)]
    return np.concatenate(outs, axis=0)[:n].astype(np.float32)


# revision 24
# speedup vs baseline: 1.5378x; 1.0015x over previous
"""GCN encoder (2-layer, PyG GCNConv w/ self-loops + symmetric norm) on 8 trn2 cores.

Math per layer: out = dis * ((A+I)(dis*x)) @ W + b, with dis = deg^-1/2.
  x' = dis * x                         (row scale, bf16)
  agg[d] = sum_{edges s->d} x'[s]      (dma_gather + PE segment-sum vs one-hot S)
  y[d] = f(dis[d] * agg[d] @ W + b)    (f = relu for layer 1)
Sharding: destination nodes row-sharded across 8 cores (49 blocks of 128 each);
x' replicated via redundant prep (layer 1) and two pipelined AllGathers
(layer 2, A=28-block half / B=21-block half so gathers overlap the 2nd AG).
"""

import sys

sys.path.insert(0, "/opt/trn_rl_repo")

import numpy as np
import ml_dtypes

BF16 = ml_dtypes.bfloat16

D = 128
P = 8
BA = 28  # blocks per core in the "A" half (must be a multiple of group size)
GS = 7  # dest blocks per gather group


def _sizes(n):
    rpc = -(-n // (P * 128)) * 128  # rows per core, multiple of 128
    npad = rpc * P
    b = rpc // 128  # dest blocks per core
    nt = npad // 128
    gs = 1
    for d_ in range(1, 9):
        if b % d_ == 0:
            gs = d_
    g = b // gs
    ba = ((g + 1) // 2) * gs if g >= 2 else b  # A-half blocks, group-aligned (B smaller, AG_B first)
    ra, rb = ba * 128, (b - ba) * 128
    return rpc, npad, b, nt, gs, ba, ra, rb


def plan(edge_index, n):
    """Host-side integer preprocessing.

    Destinations are permuted into degree-balanced 128-row blocks (round-robin
    over blocks by descending degree) so every (block, src-half) has a near-
    equal edge count -> minimal chunk padding. Layer 1 gathers from the
    original-order x'; layer 2 gathers from the permuted-order activations,
    so each layer gets its own index/drel tables.
    """
    rpc, npad, b, nt, gs, ba, ra, rb = _sizes(n)
    nblocks = P * b
    src = edge_index[0].astype(np.int64)
    dst = edge_index[1].astype(np.int64)
    loops = np.arange(n, dtype=np.int64)
    allsrc = np.concatenate([src, loops])
    alldst = np.concatenate([dst, loops])

    deg = np.bincount(alldst, minlength=n).astype(np.float32)
    deg_pad = np.ones(npad, dtype=np.float32)
    deg_pad[:n] = deg
    deg_t = np.ascontiguousarray(deg_pad.reshape(nt, 128).T)  # [128, nt] orig order

    # degree-balanced destination permutation: node -> padded row
    by_deg = np.argsort(-deg, kind="stable")
    bid = np.arange(n, dtype=np.int64) % nblocks
    slot = np.arange(n, dtype=np.int64) // nblocks
    perm_row = np.empty(n, dtype=np.int64)
    perm_row[by_deg] = (bid // b) * rpc + (bid % b) * 128 + slot
    degrow = np.ones(npad, dtype=np.float32)
    degrow[perm_row] = deg
    degrow_t = np.ascontiguousarray(degrow.reshape(nt, 128).T)  # [128, nt] permuted

    dst_row = perm_row[alldst]
    core = dst_row // rpc
    dloc = dst_row - core * rpc
    blk = dloc >> 7
    drel = (dloc & 127).astype(np.float32)

    # per-layer source row mappings into the A/B halves
    s_core1 = allsrc // rpc
    s_w1 = allsrc - s_core1 * rpc
    hi1 = (s_w1 >= ra).astype(np.int64)
    sidx1 = np.where(hi1 == 0, s_core1 * ra + s_w1, s_core1 * rb + (s_w1 - ra))
    src_row2 = perm_row[allsrc]
    s_core2 = src_row2 // rpc
    s_w2 = src_row2 - s_core2 * rpc
    hi2 = (s_w2 >= ra).astype(np.int64)
    sidx2 = np.where(hi2 == 0, s_core2 * ra + s_w2, s_core2 * rb + (s_w2 - ra))
    assert sidx1.max() < 32768 and sidx2.max() < 32768

    g = b // gs
    nkeys = P * b * 2

    def slotize(sidx, hi):
        key = (core * b + blk) * 2 + hi
        counts = np.bincount(key, minlength=nkeys)
        cc = counts.reshape(P, b, 2)
        k2lo = max(1, int(-(-cc[:, :, 0].max() // 128)))
        k2hi = max(1, int(-(-cc[:, :, 1].max() // 128)))
        spl, sph = k2lo * 128, k2hi * 128
        order = np.argsort(key, kind="stable")
        key_s = key[order]
        run_start = np.zeros(nkeys, dtype=np.int64)
        np.cumsum(counts[:-1], out=run_start[1:])
        rank = np.arange(key_s.size, dtype=np.int64) - run_start[key_s]
        slot_base = np.zeros(nkeys + 1, dtype=np.int64)
        per_key = np.where(np.arange(nkeys) % 2 == 0, spl, sph)
        np.cumsum(per_key, out=slot_base[1:])
        pos = slot_base[key_s] + rank
        tot = int(slot_base[-1])
        idx_flat = np.zeros(tot, dtype=np.int16)
        drel_flat = np.full(tot, -1.0, dtype=np.float32)
        idx_flat[pos] = sidx[order].astype(np.int16)
        drel_flat[pos] = drel[order]
        per_blk = idx_flat.reshape(P, b, spl + sph)
        drel_blk = drel_flat.reshape(P, b, spl + sph)
        k2t = k2lo + k2hi
        L_lo, L_hi = gs * spl, gs * sph

        def make_idx(core_slots, L):
            l16 = L // 16
            seq = core_slots.reshape(g, L)  # [g, L] block-major
            tile = seq.reshape(g, l16, 16).transpose(0, 2, 1)
            tile = np.tile(tile, (1, 8, 1))
            return np.ascontiguousarray(
                tile.transpose(1, 0, 2).reshape(128, g * l16)
            )

        idx_lo = [make_idx(per_blk[c, :, :spl], L_lo) for c in range(P)]
        idx_hi = [make_idx(per_blk[c, :, spl:], L_hi) for c in range(P)]
        drs = []
        for c in range(P):
            dr = drel_blk[c].reshape(b, k2t, 128).transpose(2, 0, 1)
            drs.append(np.ascontiguousarray(dr.reshape(128, b * k2t)).astype(BF16))
        return dict(
            k2lo=k2lo, k2hi=k2hi, k2t=k2t, l16lo=L_lo // 16, l16hi=L_hi // 16,
            idx_lo=idx_lo, idx_hi=idx_hi, drel=drs,
        )

    lay1 = slotize(sidx1, hi1)
    lay2 = slotize(sidx2, hi2)
    k2tmax = max(lay1["k2t"], lay2["k2t"])

    per_core = []
    for c in range(P):
        deg_own = np.ascontiguousarray(degrow_t[:, c * b : (c + 1) * b])
        per_core.append(
            {
                "idx_lo1": lay1["idx_lo"][c], "idx_hi1": lay1["idx_hi"][c],
                "idx_lo2": lay2["idx_lo"][c], "idx_hi2": lay2["idx_hi"][c],
                "drel1": lay1["drel"][c], "drel2": lay2["drel"][c],
                "deg_own": deg_own,
            }
        )

    # iota_rep[p, j*k2tmax + c] = j  (chunk-minor layout for 2x-mode is_equal)
    iota_rep = np.repeat(np.arange(128, dtype=np.float32), k2tmax)
    iota_rep = np.tile(iota_rep, (128, 1)).astype(BF16)
    ident = np.eye(128, dtype=np.float32)
    lk = tuple(
        (la["k2lo"], la["k2hi"], la["l16lo"], la["l16hi"]) for la in (lay1, lay2)
    )
    return {
        "sizes": (rpc, npad, b, nt, gs, ba, ra, rb, g, k2tmax) + lk,
        "deg_t": deg_t,
        "per_core": per_core,
        "perm_row": perm_row,
        "iota_rep": iota_rep,
        "ident_bf": ident.astype(BF16),
        "ident_f32": ident,
    }


def build_program(pl):
    import concourse.mybir as mybir
    from concourse.bacc import Bacc
    from concourse.tile import TileContext

    (rpc, npad, b, nt, gs, ba, ra, rb, g, k2tmax, lk1, lk2) = pl["sizes"]
    na, nb = P * ra, P * rb
    ga = ba // gs  # groups in the A half
    f32 = mybir.dt.float32
    bf16 = mybir.dt.bfloat16
    i16 = mybir.dt.int16
    AF = mybir.ActivationFunctionType
    OP = mybir.AluOpType

    nc = Bacc(num_devices=P)

    x_in = nc.declare_dram_parameter("x", [npad, D], f32, isOutput=False)
    degt_in = nc.declare_dram_parameter("deg_t", [128, nt], f32, isOutput=False)
    dego_in = nc.declare_dram_parameter("deg_own", [128, b], f32, isOutput=False)
    w1_in = nc.declare_dram_parameter("W1", [D, D], f32, isOutput=False)
    b1_in = nc.declare_dram_parameter("b1", [D, 1], f32, isOutput=False)
    w2_in = nc.declare_dram_parameter("W2", [D, D], f32, isOutput=False)
    b2_in = nc.declare_dram_parameter("b2", [D, 1], f32, isOutput=False)
    b2t_in = nc.declare_dram_parameter("b2_tile", [D, D], f32, isOutput=False)
    iota_in = nc.declare_dram_parameter("iota_rep", [128, 128 * k2tmax], bf16, isOutput=False)
    identb_in = nc.declare_dram_parameter("ident_bf", [128, 128], bf16, isOutput=False)
    identf_in = nc.declare_dram_parameter("ident_f32", [128, 128], f32, isOutput=False)
    lay_in = []
    for li, (k2lo, k2hi, l16lo, l16hi) in ((1, lk1), (2, lk2)):
        lay_in.append(
            (
                nc.declare_dram_parameter(
                    f"idx_lo{li}", [128, g * l16lo], i16, isOutput=False
                ),
                nc.declare_dram_parameter(
                    f"idx_hi{li}", [128, g * l16hi], i16, isOutput=False
                ),
                nc.declare_dram_parameter(
                    f"drel{li}", [128, b * (k2lo + k2hi)], bf16, isOutput=False
                ),
            )
        )
    out = nc.declare_dram_parameter("out", [rpc, D], f32, isOutput=True)

    split = rb > 0
    x1a = nc.dram_tensor("x1a", [na, D], bf16)
    x2own_a = nc.dram_tensor("x2own_a", [ra, D], bf16)
    x2lo = nc.dram_tensor("x2lo", [na, D], bf16, addr_space="Shared")
    if split:
        x1b = nc.dram_tensor("x1b", [nb, D], bf16)
        x2own_b = nc.dram_tensor("x2own_b", [rb, D], bf16)
        x2hi = nc.dram_tensor("x2hi", [nb, D], bf16, addr_space="Shared")
    else:
        x1b, x2own_b, x2hi = x1a, None, x2lo

    with TileContext(nc) as tc:
        with (
            tc.tile_pool(name="const", bufs=1) as const,
            tc.tile_pool(name="prep", bufs=4) as prep,
            tc.tile_pool(name="msgs", bufs=3) as msgs,
            tc.tile_pool(name="spool", bufs=3) as spool,
            tc.tile_pool(name="epi", bufs=4) as epi,
            tc.tile_pool(name="pa", bufs=3, space="PSUM") as pa,
            tc.tile_pool(name="pt", bufs=3, space="PSUM") as pt,
            tc.tile_pool(name="pz", bufs=2, space="PSUM") as pz,
        ):
            # ---- constants -------------------------------------------------
            def load_const(param, shape, dtype, tag):
                t = const.tile(shape, dtype, tag=tag)
                nc.sync.dma_start(t[:], param[:])
                return t

            degt_sb = load_const(degt_in, [128, nt], f32, "degt")
            dego_sb = load_const(dego_in, [128, b], f32, "dego")
            w1_sb = load_const(w1_in, [D, D], f32, "w1")
            w2_sb = load_const(w2_in, [D, D], f32, "w2")
            b1_sb = load_const(b1_in, [D, 1], f32, "b1")
            b2_sb = load_const(b2_in, [D, 1], f32, "b2")
            b2t_sb = load_const(b2t_in, [D, D], f32, "b2t")
            iota_sb = load_const(iota_in, [128, 128 * k2tmax], bf16, "iota")
            identb_sb = load_const(identb_in, [128, 128], bf16, "identb")
            identf_sb = load_const(identf_in, [128, 128], f32, "identf")
            lay_sb = []
            for li, (k2lo, k2hi, l16lo, l16hi) in ((0, lk1), (1, lk2)):
                ilo, ihi, drl = lay_in[li]
                lay_sb.append(
                    (
                        load_const(ilo, [128, g * l16lo], i16, f"idxlo{li}"),
                        load_const(ihi, [128, g * l16hi], i16, f"idxhi{li}"),
                        load_const(drl, [128, b * (k2lo + k2hi)], bf16, f"drel{li}"),
                    )
                )

            rec_t = const.tile([128, nt], f32, tag="rec_t")
            nc.vector.reciprocal(rec_t[:], degt_sb[:])
            dis_t = const.tile([128, nt], f32, tag="dis_t")
            nc.scalar.activation(dis_t[:], rec_t[:], AF.Sqrt)
            rec_o = const.tile([128, b], f32, tag="rec_o")
            nc.vector.reciprocal(rec_o[:], dego_sb[:])
            dis_o = const.tile([128, b], f32, tag="dis_o")
            nc.scalar.activation(dis_o[:], rec_o[:], AF.Sqrt)

            w1b = const.tile([D, D], bf16, tag="w1b")
            nc.vector.tensor_copy(w1b[:], w1_sb[:])
            w2b = const.tile([D, D], bf16, tag="w2b")
            nc.vector.tensor_copy(w2b[:], w2_sb[:])

            # ---- prep: x1{a,b} = bf16(x * dis), batched --------------------
            def prep_run(tile0, ntiles, dstt, drow0):
                # process `ntiles` consecutive 128-row tiles starting at
                # global tile `tile0`, writing to dstt rows starting drow0
                off = 0
                while off < ntiles:
                    ch = min(14, ntiles - off)
                    t0 = tile0 + off
                    xt = prep.tile([128, 14, D], f32, tag="xt")
                    nc.sync.dma_start(
                        xt[:, 0:ch, :],
                        x_in[t0 * 128 : (t0 + ch) * 128, :].rearrange(
                            "(a p) d -> p a d", p=128
                        ),
                    )
                    xb = prep.tile([128, 14, D], bf16, tag="xb")
                    for i in range(ch):
                        nc.scalar.activation(
                            xb[:, i, :],
                            xt[:, i, :],
                            AF.Copy,
                            scale=dis_t[:, t0 + i : t0 + i + 1],
                        )
                    r0 = drow0 + off * 128
                    nc.sync.dma_start(
                        dstt[r0 : r0 + ch * 128, :].rearrange("(a p) d -> p a d", p=128),
                        xb[:, 0:ch, :],
                    )
                    off += ch

            for sc in range(P):
                prep_run(sc * b, ba, x1a, sc * ra)
            if split:
                for sc in range(P):
                    prep_run(sc * b + ba, b - ba, x1b, sc * rb)

            # ---- one GCN layer, two source-half phases --------------------
            # phase A accumulates dis[d]*sum(msgs of one half) into partial;
            # phase B adds the other half, then runs the block epilogue.
            partial = const.tile([128, b * 128], f32, tag="partial")

            def halves(lay, h):
                k2lo, k2hi, l16lo, l16hi = (lk1, lk2)[lay]
                ilo, ihi, drl = lay_sb[lay]
                k2t = k2lo + k2hi
                if h == 0:
                    return ilo, 16 * l16lo, l16lo, k2lo, 0, k2t, drl
                return ihi, 16 * l16hi, l16hi, k2hi, k2lo, k2t, drl

            def gather_half(gg, src, lay, h):
                idx_sb, L, l16, k2h, _, _, _ = halves(lay, h)
                msg = msgs.tile([128, gs * k2h, D], bf16, tag="msg")
                nc.gpsimd.dma_gather(
                    msg[:, :, :],
                    src,
                    idx_sb[:, gg * l16 : (gg + 1) * l16],
                    L,
                    L,
                    D,
                    single_packet=False,
                )
                return msg

            def block_agg(j, bb, msg, lay, h):
                _, _, _, k2h, koff, k2t, drel_sb = halves(lay, h)
                S = spool.tile([128, 128, k2tmax], bf16, tag="S")
                nc.vector.tensor_tensor(
                    S[:, :, 0:k2h],
                    iota_sb[:, :]
                    .rearrange("p (j c) -> p j c", j=128)[:, :, 0:k2h],
                    drel_sb[:, bb * k2t + koff : bb * k2t + koff + k2h]
                    .rearrange("p (a c) -> p a c", a=1)
                    .broadcast_to([128, 128, k2h]),
                    OP.is_equal,
                )
                agg = pa.tile([128, D], f32, tag="agg")
                for k in range(k2h):
                    nc.tensor.matmul(
                        agg[:],
                        S[:, :, k],
                        msg[:, j * k2h + k, :],
                        start=(k == 0),
                        stop=(k == k2h - 1),
                    )
                return agg

            def do_phase_a(gg, src, lay, h):
                msg = gather_half(gg, src, lay, h)
                for j in range(gs):
                    bb = gg * gs + j
                    agg = block_agg(j, bb, msg, lay, h)
                    nc.scalar.activation(
                        partial[:, bb * 128 : (bb + 1) * 128],
                        agg[:],
                        AF.Copy,
                        scale=dis_o[:, bb : bb + 1],
                    )

            def do_phase_b(gg, src, lay, h, first):
                wb = w1b if first else w2b
                msg = gather_half(gg, src, lay, h)
                for j in range(gs):
                    bb = gg * gs + j
                    agg = block_agg(j, bb, msg, lay, h)
                    # aggs = dis[d]*agg + partial  (bf16)
                    aggs = epi.tile([128, D], bf16, tag="aggs")
                    nc.vector.scalar_tensor_tensor(
                        aggs[:],
                        agg[:],
                        dis_o[:, bb : bb + 1],
                        partial[:, bb * 128 : (bb + 1) * 128],
                        OP.mult,
                        OP.add,
                    )
                    aggT_p = pt.tile([128, D], bf16, tag="aggT_p")
                    nc.tensor.transpose(aggT_p[:], aggs[:], identb_sb[:])
                    aggT = epi.tile([128, D], bf16, tag="aggT")
                    nc.scalar.activation(aggT[:], aggT_p[:], AF.Copy)
                    if first:
                        z_p = pz.tile([128, D], f32, tag="z_p")
                        nc.tensor.matmul(
                            z_p[:], wb[:], aggT[:], start=True, stop=True
                        )
                        zs = epi.tile([128, D], bf16, tag="zs")
                        nc.scalar.activation(
                            zs[:], z_p[:], AF.Relu, bias=b1_sb[:, 0:1]
                        )
                        y_p = pz.tile([128, D], bf16, tag="z_p")
                        nc.tensor.transpose(y_p[:], zs[:], identb_sb[:])
                        ys = epi.tile([128, D], bf16, tag="ys")
                        nc.vector.tensor_scalar(
                            ys[:], y_p[:], dis_o[:, bb : bb + 1], None, OP.mult
                        )
                        if bb < ba:
                            nc.sync.dma_start(
                                x2own_a[bb * 128 : (bb + 1) * 128, :], ys[:]
                            )
                        else:
                            r0 = (bb - ba) * 128
                            nc.sync.dma_start(x2own_b[r0 : r0 + 128, :], ys[:])
                    else:
                        # direct [dest, dhid] = aggT.T @ W, then + b2 tile
                        z_p = pz.tile([128, D], f32, tag="z_p")
                        nc.tensor.matmul(
                            z_p[:], aggT[:], wb[:], start=True, stop=True
                        )
                        ys = epi.tile([128, D], f32, tag="ys2")
                        nc.vector.scalar_tensor_tensor(
                            ys[:], z_p[:], 1.0, b2t_sb[:], OP.mult, OP.add
                        )
                        nc.sync.dma_start(out[bb * 128 : (bb + 1) * 128, :], ys[:])

            # layer 1: B-half dest groups first so AG_B can start early
            for gg in range(ga, g):
                do_phase_a(gg, x1a[:, :], 0, 0)
            for gg in range(ga, g):
                do_phase_b(gg, x1b[:, :], 0, 1, True)
            if split:
                nc.gpsimd.collective_compute(
                    "AllGather",
                    mybir.AluOpType.bypass,
                    replica_groups=[list(range(P))],
                    ins=[x2own_b[:]],
                    outs=[x2hi[:]],
                )
            for gg in range(ga):
                do_phase_a(gg, x1a[:, :], 0, 0)
            for gg in range(ga):
                do_phase_b(gg, x1b[:, :], 0, 1, True)
            nc.gpsimd.collective_compute(
                "AllGather",
                mybir.AluOpType.bypass,
                replica_groups=[list(range(P))],
                ins=[x2own_a[:]],
                outs=[x2lo[:]],
            )
            # layer 2: hi phase first (needs only AG_B), lo merge phase after AG_A
            for gg in range(g):
                do_phase_a(gg, x2hi[:, :], 1, 1)
            for gg in range(g):
                do_phase_b(gg, x2lo[:, :], 1, 0, False)

    nc.finalize()
    return nc


def make_in_maps(pl, x, w1, b1, w2, b2):
    n = x.shape[0]
    npad = pl["sizes"][1]
    x_pad = np.zeros((npad, D), dtype=np.float32)
    x_pad[:n] = x
    shared = {
        "x": x_pad,
        "deg_t": pl["deg_t"],
        "W1": np.ascontiguousarray(w1.astype(np.float32)),
        "b1": np.ascontiguousarray(b1.astype(np.float32).reshape(D, 1)),
        "W2": np.ascontiguousarray(w2.astype(np.float32)),
        "b2": np.ascontiguousarray(b2.astype(np.float32).reshape(D, 1)),
        "b2_tile": np.ascontiguousarray(
            np.tile(b2.astype(np.float32).reshape(1, D), (D, 1))
        ),
        "iota_rep": pl["iota_rep"],
        "ident_bf": pl["ident_bf"],
        "ident_f32": pl["ident_f32"],
    }
    in_maps = []
    for c in range(P):
        m = dict(shared)
        pc = pl["per_core"]
        for kk in ("deg_own", "idx_lo1", "idx_hi1", "idx_lo2", "idx_hi2",
                   "drel1", "drel2"):
            m[kk] = pc[c][kk]
        in_maps.append(m)
    return in_maps


_CACHE = {}


def kernel(x, edge_index, W1, b1, W2, b2):
    from concourse.bass_utils import run_bass_kernel_spmd

    x = np.asarray(x)
    edge_index = np.asarray(edge_index)
    n = x.shape[0]
    pl = plan(edge_index, n)
    key = pl["sizes"]
    if key not in _CACHE:
        _CACHE[key] = build_program(pl)
    nc = _CACHE[key]
    in_maps = make_in_maps(
        pl, x, np.asarray(W1), np.asarray(b1), np.asarray(W2), np.asarray(b2)
    )
    r = run_bass_kernel_spmd(nc, in_maps, list(range(P)))
    outs = np.concatenate([r.results[c]["out"] for c in range(P)], axis=0)
    return np.ascontiguousarray(outs[pl["perm_row"][:n]]).astype(np.float32)


# revision 29
# speedup vs baseline: 1.6139x; 1.0494x over previous
"""GCN encoder (2-layer, PyG GCNConv w/ self-loops + symmetric norm) on 8 trn2 cores.

Math per layer: out = dis * ((A+I)(dis*x)) @ W + b, with dis = deg^-1/2.
  x' = dis * x                         (row scale, bf16)
  agg[d] = sum_{edges s->d} x'[s]      (dma_gather + PE segment-sum vs one-hot S)
  y[d] = f(dis[d] * agg[d] @ W + b)    (f = relu for layer 1)
Sharding: destination nodes row-sharded across 8 cores (49 blocks of 128 each);
x' replicated via redundant prep (layer 1) and two pipelined AllGathers
(layer 2, A=28-block half / B=21-block half so gathers overlap the 2nd AG).
"""

import sys

sys.path.insert(0, "/opt/trn_rl_repo")

import numpy as np
import ml_dtypes

BF16 = ml_dtypes.bfloat16

D = 128
P = 8
BA = 28  # blocks per core in the "A" half (must be a multiple of group size)
GS = 7  # dest blocks per gather group


def _sizes(n):
    rpc = -(-n // (P * 128)) * 128  # rows per core, multiple of 128
    npad = rpc * P
    b = rpc // 128  # dest blocks per core
    nt = npad // 128
    gs = 1
    for d_ in range(1, 9):
        if b % d_ == 0:
            gs = d_
    g = b // gs
    ba = ((g + 1) // 2) * gs if g >= 2 else b  # A-half blocks, group-aligned
    ra, rb = ba * 128, (b - ba) * 128
    return rpc, npad, b, nt, gs, ba, ra, rb


def plan(edge_index, n):
    """Host-side integer preprocessing.

    Destinations are permuted into degree-balanced 128-row blocks (round-robin
    over blocks by descending degree) so every (block, src-half) has a near-
    equal edge count -> minimal chunk padding. Layer 1 gathers from the
    original-order x'; layer 2 gathers from the permuted-order activations,
    so each layer gets its own index/drel tables.
    """
    rpc, npad, b, nt, gs, ba, ra, rb = _sizes(n)
    nblocks = P * b
    src = edge_index[0].astype(np.int64)
    dst = edge_index[1].astype(np.int64)
    loops = np.arange(n, dtype=np.int64)
    allsrc = np.concatenate([src, loops])
    alldst = np.concatenate([dst, loops])

    deg = np.bincount(alldst, minlength=n).astype(np.float32)
    deg_pad = np.ones(npad, dtype=np.float32)
    deg_pad[:n] = deg
    deg_t = np.ascontiguousarray(deg_pad.reshape(nt, 128).T)  # [128, nt] orig order

    # degree-balanced destination permutation: node -> padded row
    by_deg = np.argsort(-deg, kind="stable")
    bid = np.arange(n, dtype=np.int64) % nblocks
    slot = np.arange(n, dtype=np.int64) // nblocks
    perm_row = np.empty(n, dtype=np.int64)
    perm_row[by_deg] = (bid // b) * rpc + (bid % b) * 128 + slot
    degrow = np.ones(npad, dtype=np.float32)
    degrow[perm_row] = deg
    degrow_t = np.ascontiguousarray(degrow.reshape(nt, 128).T)  # [128, nt] permuted

    dst_row = perm_row[alldst]
    core = dst_row // rpc
    dloc = dst_row - core * rpc
    blk = dloc >> 7
    drel = (dloc & 127).astype(np.float32)

    # per-layer source row mappings into the A/B halves
    s_core1 = allsrc // rpc
    s_w1 = allsrc - s_core1 * rpc
    hi1 = (s_w1 >= ra).astype(np.int64)
    sidx1 = np.where(hi1 == 0, s_core1 * ra + s_w1, s_core1 * rb + (s_w1 - ra))
    src_row2 = perm_row[allsrc]
    s_core2 = src_row2 // rpc
    s_w2 = src_row2 - s_core2 * rpc
    hi2 = (s_w2 >= ra).astype(np.int64)
    sidx2 = np.where(hi2 == 0, s_core2 * ra + s_w2, s_core2 * rb + (s_w2 - ra))
    assert sidx1.max() < 32768 and sidx2.max() < 32768

    g = b // gs
    nkeys = P * b * 2

    def slotize(sidx, hi):
        key = (core * b + blk) * 2 + hi
        counts = np.bincount(key, minlength=nkeys)
        cc = counts.reshape(P, b, 2)
        k2lo = max(1, int(-(-cc[:, :, 0].max() // 128)))
        k2hi = max(1, int(-(-cc[:, :, 1].max() // 128)))
        spl, sph = k2lo * 128, k2hi * 128
        order = np.argsort(key, kind="stable")
        key_s = key[order]
        run_start = np.zeros(nkeys, dtype=np.int64)
        np.cumsum(counts[:-1], out=run_start[1:])
        rank = np.arange(key_s.size, dtype=np.int64) - run_start[key_s]
        slot_base = np.zeros(nkeys + 1, dtype=np.int64)
        per_key = np.where(np.arange(nkeys) % 2 == 0, spl, sph)
        np.cumsum(per_key, out=slot_base[1:])
        pos = slot_base[key_s] + rank
        tot = int(slot_base[-1])
        idx_flat = np.zeros(tot, dtype=np.int16)
        drel_flat = np.full(tot, -1.0, dtype=np.float32)
        idx_flat[pos] = sidx[order].astype(np.int16)
        drel_flat[pos] = drel[order]
        per_blk = idx_flat.reshape(P, b, spl + sph)
        drel_blk = drel_flat.reshape(P, b, spl + sph)
        k2t = k2lo + k2hi
        L_lo, L_hi = gs * spl, gs * sph

        def make_idx(core_slots, L):
            l16 = L // 16
            seq = core_slots.reshape(g, L)  # [g, L] block-major
            tile = seq.reshape(g, l16, 16).transpose(0, 2, 1)
            tile = np.tile(tile, (1, 8, 1))
            return np.ascontiguousarray(
                tile.transpose(1, 0, 2).reshape(128, g * l16)
            )

        idx_lo = [make_idx(per_blk[c, :, :spl], L_lo) for c in range(P)]
        idx_hi = [make_idx(per_blk[c, :, spl:], L_hi) for c in range(P)]
        drs = []
        for c in range(P):
            dr = drel_blk[c].reshape(b, k2t, 128).transpose(2, 0, 1)
            drs.append(np.ascontiguousarray(dr.reshape(128, b * k2t)).astype(BF16))
        return dict(
            k2lo=k2lo, k2hi=k2hi, k2t=k2t, l16lo=L_lo // 16, l16hi=L_hi // 16,
            idx_lo=idx_lo, idx_hi=idx_hi, drel=drs,
        )

    lay1 = slotize(sidx1, hi1)
    lay2 = slotize(sidx2, hi2)
    k2tmax = max(lay1["k2t"], lay2["k2t"])

    per_core = []
    for c in range(P):
        deg_own = np.ascontiguousarray(degrow_t[:, c * b : (c + 1) * b])
        per_core.append(
            {
                "idx_lo1": lay1["idx_lo"][c], "idx_hi1": lay1["idx_hi"][c],
                "idx_lo2": lay2["idx_lo"][c], "idx_hi2": lay2["idx_hi"][c],
                "drel1": lay1["drel"][c], "drel2": lay2["drel"][c],
                "deg_own": deg_own,
            }
        )

    # iota_rep[p, j*k2tmax + c] = j  (chunk-minor layout for 2x-mode is_equal)
    iota_rep = np.repeat(np.arange(128, dtype=np.float32), k2tmax)
    iota_rep = np.tile(iota_rep, (128, 1)).astype(BF16)
    ident = np.eye(128, dtype=np.float32)
    lk = tuple(
        (la["k2lo"], la["k2hi"], la["l16lo"], la["l16hi"]) for la in (lay1, lay2)
    )
    return {
        "sizes": (rpc, npad, b, nt, gs, ba, ra, rb, g, k2tmax) + lk,
        "deg_t": deg_t,
        "per_core": per_core,
        "perm_row": perm_row,
        "iota_rep": iota_rep,
        "ident_bf": ident.astype(BF16),
        "ident_f32": ident,
    }


def build_program(pl):
    import concourse.mybir as mybir
    from concourse.bacc import Bacc
    from concourse.tile import TileContext

    (rpc, npad, b, nt, gs, ba, ra, rb, g, k2tmax, lk1, lk2) = pl["sizes"]
    na, nb = P * ra, P * rb
    ga = ba // gs  # groups in the A half
    f32 = mybir.dt.float32
    bf16 = mybir.dt.bfloat16
    i16 = mybir.dt.int16
    AF = mybir.ActivationFunctionType
    OP = mybir.AluOpType

    nc = Bacc(num_devices=P)

    x_in = nc.declare_dram_parameter("x", [npad, D], f32, isOutput=False)
    degt_in = nc.declare_dram_parameter("deg_t", [128, nt], f32, isOutput=False)
    dego_in = nc.declare_dram_parameter("deg_own", [128, b], f32, isOutput=False)
    w1_in = nc.declare_dram_parameter("W1", [D, D], f32, isOutput=False)
    b1_in = nc.declare_dram_parameter("b1", [D, 1], f32, isOutput=False)
    w2_in = nc.declare_dram_parameter("W2", [D, D], f32, isOutput=False)
    b2_in = nc.declare_dram_parameter("b2", [D, 1], f32, isOutput=False)
    b2t_in = nc.declare_dram_parameter("b2_tile", [D, D], f32, isOutput=False)
    iota_in = nc.declare_dram_parameter("iota_rep", [128, 128 * k2tmax], bf16, isOutput=False)
    identb_in = nc.declare_dram_parameter("ident_bf", [128, 128], bf16, isOutput=False)
    identf_in = nc.declare_dram_parameter("ident_f32", [128, 128], f32, isOutput=False)
    lay_in = []
    for li, (k2lo, k2hi, l16lo, l16hi) in ((1, lk1), (2, lk2)):
        lay_in.append(
            (
                nc.declare_dram_parameter(
                    f"idx_lo{li}", [128, g * l16lo], i16, isOutput=False
                ),
                nc.declare_dram_parameter(
                    f"idx_hi{li}", [128, g * l16hi], i16, isOutput=False
                ),
                nc.declare_dram_parameter(
                    f"drel{li}", [128, b * (k2lo + k2hi)], bf16, isOutput=False
                ),
            )
        )
    out = nc.declare_dram_parameter("out", [rpc, D], f32, isOutput=True)

    split = rb > 0
    x1a = nc.dram_tensor("x1a", [na, D], bf16)
    x2own_a = nc.dram_tensor("x2own_a", [ra, D], bf16)
    x2lo = nc.dram_tensor("x2lo", [na, D], bf16, addr_space="Shared")
    if split:
        x1b = nc.dram_tensor("x1b", [nb, D], bf16)
        x2own_b = nc.dram_tensor("x2own_b", [rb, D], bf16)
        x2hi = nc.dram_tensor("x2hi", [nb, D], bf16, addr_space="Shared")
    else:
        x1b, x2own_b, x2hi = x1a, None, x2lo

    with TileContext(nc) as tc:
        with (
            tc.tile_pool(name="const", bufs=1) as const,
            tc.tile_pool(name="prep", bufs=4) as prep,
            tc.tile_pool(name="msgs", bufs=3) as msgs,
            tc.tile_pool(name="spool", bufs=4) as spool,
            tc.tile_pool(name="epi", bufs=8) as epi,
            tc.tile_pool(name="pa", bufs=4, space="PSUM") as pa,
            tc.tile_pool(name="pt", bufs=2, space="PSUM") as pt,
            tc.tile_pool(name="pz", bufs=2, space="PSUM") as pz,
        ):
            # ---- constants -------------------------------------------------
            def load_const(param, shape, dtype, tag):
                t = const.tile(shape, dtype, tag=tag)
                nc.sync.dma_start(t[:], param[:])
                return t

            degt_sb = load_const(degt_in, [128, nt], f32, "degt")
            dego_sb = load_const(dego_in, [128, b], f32, "dego")
            w1_sb = load_const(w1_in, [D, D], f32, "w1")
            w2_sb = load_const(w2_in, [D, D], f32, "w2")
            b1_sb = load_const(b1_in, [D, 1], f32, "b1")
            b2_sb = load_const(b2_in, [D, 1], f32, "b2")
            b2t_sb = load_const(b2t_in, [D, D], f32, "b2t")
            iota_sb = load_const(iota_in, [128, 128 * k2tmax], bf16, "iota")
            identb_sb = load_const(identb_in, [128, 128], bf16, "identb")
            identf_sb = load_const(identf_in, [128, 128], f32, "identf")
            lay_sb = []
            for li, (k2lo, k2hi, l16lo, l16hi) in ((0, lk1), (1, lk2)):
                ilo, ihi, drl = lay_in[li]
                lay_sb.append(
                    (
                        load_const(ilo, [128, g * l16lo], i16, f"idxlo{li}"),
                        load_const(ihi, [128, g * l16hi], i16, f"idxhi{li}"),
                        load_const(drl, [128, b * (k2lo + k2hi)], bf16, f"drel{li}"),
                    )
                )

            rec_t = const.tile([128, nt], f32, tag="rec_t")
            nc.vector.reciprocal(rec_t[:], degt_sb[:])
            dis_t = const.tile([128, nt], f32, tag="dis_t")
            nc.scalar.activation(dis_t[:], rec_t[:], AF.Sqrt)
            rec_o = const.tile([128, b], f32, tag="rec_o")
            nc.vector.reciprocal(rec_o[:], dego_sb[:])
            dis_o = const.tile([128, b], f32, tag="dis_o")
            nc.scalar.activation(dis_o[:], rec_o[:], AF.Sqrt)

            w1b = const.tile([D, D], bf16, tag="w1b")
            nc.vector.tensor_copy(w1b[:], w1_sb[:])
            w2b = const.tile([D, D], bf16, tag="w2b")
            nc.vector.tensor_copy(w2b[:], w2_sb[:])

            # ---- prep: x1{a,b} = bf16(x * dis), batched --------------------
            def prep_run(tile0, ntiles, dstt, drow0):
                # process `ntiles` consecutive 128-row tiles starting at
                # global tile `tile0`, writing to dstt rows starting drow0
                off = 0
                while off < ntiles:
                    ch = min(14, ntiles - off)
                    t0 = tile0 + off
                    xt = prep.tile([128, 14, D], f32, tag="xt")
                    nc.sync.dma_start(
                        xt[:, 0:ch, :],
                        x_in[t0 * 128 : (t0 + ch) * 128, :].rearrange(
                            "(a p) d -> p a d", p=128
                        ),
                    )
                    xb = prep.tile([128, 14, D], bf16, tag="xb")
                    for i in range(ch):
                        nc.scalar.activation(
                            xb[:, i, :],
                            xt[:, i, :],
                            AF.Copy,
                            scale=dis_t[:, t0 + i : t0 + i + 1],
                        )
                    r0 = drow0 + off * 128
                    nc.sync.dma_start(
                        dstt[r0 : r0 + ch * 128, :].rearrange("(a p) d -> p a d", p=128),
                        xb[:, 0:ch, :],
                    )
                    off += ch

            for sc in range(P):
                prep_run(sc * b, ba, x1a, sc * ra)
            if split:
                for sc in range(P):
                    prep_run(sc * b + ba, b - ba, x1b, sc * rb)

            # ---- one GCN layer, two source-half phases --------------------
            # phase A accumulates dis[d]*sum(msgs of one half) into partial;
            # phase B adds the other half, then runs the block epilogue.
            partial = const.tile([128, b * 128], f32, tag="partial")

            def halves(lay, h):
                k2lo, k2hi, l16lo, l16hi = (lk1, lk2)[lay]
                ilo, ihi, drl = lay_sb[lay]
                k2t = k2lo + k2hi
                if h == 0:
                    return ilo, 16 * l16lo, l16lo, k2lo, 0, k2t, drl
                return ihi, 16 * l16hi, l16hi, k2hi, k2lo, k2t, drl

            def gather_half(gg, src, lay, h):
                idx_sb, L, l16, k2h, _, _, _ = halves(lay, h)
                msg = msgs.tile([128, gs * k2h, D], bf16, tag="msg")
                nc.gpsimd.dma_gather(
                    msg[:, :, :],
                    src,
                    idx_sb[:, gg * l16 : (gg + 1) * l16],
                    L,
                    L,
                    D,
                    single_packet=False,
                )
                return msg

            def block_agg(j, bb, msg, lay, h):
                _, _, _, k2h, koff, k2t, drel_sb = halves(lay, h)
                S = spool.tile([128, 128, k2tmax], bf16, tag="S")
                nc.vector.tensor_tensor(
                    S[:, :, 0:k2h],
                    iota_sb[:, :]
                    .rearrange("p (j c) -> p j c", j=128)[:, :, 0:k2h],
                    drel_sb[:, bb * k2t + koff : bb * k2t + koff + k2h]
                    .rearrange("p (a c) -> p a c", a=1)
                    .broadcast_to([128, 128, k2h]),
                    OP.is_equal,
                )
                agg = pa.tile([128, D], f32, tag="agg")
                for k in range(k2h):
                    nc.tensor.matmul(
                        agg[:],
                        S[:, :, k],
                        msg[:, j * k2h + k, :],
                        start=(k == 0),
                        stop=(k == k2h - 1),
                    )
                return agg

            def do_phase_a(gg, src, lay, h):
                msg = gather_half(gg, src, lay, h)
                for j in range(gs):
                    bb = gg * gs + j
                    agg = block_agg(j, bb, msg, lay, h)
                    nc.scalar.activation(
                        partial[:, bb * 128 : (bb + 1) * 128],
                        agg[:],
                        AF.Copy,
                        scale=dis_o[:, bb : bb + 1],
                    )

            def do_phase_b(gg, src, lay, h, first):
                wb = w1b if first else w2b
                msg = gather_half(gg, src, lay, h)
                for j in range(gs):
                    bb = gg * gs + j
                    agg = block_agg(j, bb, msg, lay, h)
                    # aggs = dis[d]*agg + partial  (bf16)
                    aggs = epi.tile([128, D], bf16, tag="aggs")
                    nc.vector.scalar_tensor_tensor(
                        aggs[:],
                        agg[:],
                        dis_o[:, bb : bb + 1],
                        partial[:, bb * 128 : (bb + 1) * 128],
                        OP.mult,
                        OP.add,
                    )
                    aggT_p = pt.tile([128, D], bf16, tag="aggT_p")
                    nc.tensor.transpose(aggT_p[:], aggs[:], identb_sb[:])
                    aggT = epi.tile([128, D], bf16, tag="aggT")
                    nc.scalar.activation(aggT[:], aggT_p[:], AF.Copy)
                    if first:
                        z_p = pz.tile([128, D], f32, tag="z_p")
                        nc.tensor.matmul(
                            z_p[:], wb[:], aggT[:], start=True, stop=True
                        )
                        zs = epi.tile([128, D], bf16, tag="zs")
                        nc.scalar.activation(
                            zs[:], z_p[:], AF.Relu, bias=b1_sb[:, 0:1]
                        )
                        y_p = pz.tile([128, D], bf16, tag="z_p")
                        nc.tensor.transpose(y_p[:], zs[:], identb_sb[:])
                        ys = epi.tile([128, D], bf16, tag="ys")
                        nc.vector.tensor_scalar(
                            ys[:], y_p[:], dis_o[:, bb : bb + 1], None, OP.mult
                        )
                        if bb < ba:
                            nc.sync.dma_start(
                                x2own_a[bb * 128 : (bb + 1) * 128, :], ys[:]
                            )
                        else:
                            r0 = (bb - ba) * 128
                            nc.sync.dma_start(x2own_b[r0 : r0 + 128, :], ys[:])
                    else:
                        # direct [dest, dhid] = aggT.T @ W, then + b2 tile
                        z_p = pz.tile([128, D], f32, tag="z_p")
                        nc.tensor.matmul(
                            z_p[:], aggT[:], wb[:], start=True, stop=True
                        )
                        ys = epi.tile([128, D], f32, tag="ys2")
                        nc.vector.scalar_tensor_tensor(
                            ys[:], z_p[:], 1.0, b2t_sb[:], OP.mult, OP.add
                        )
                        nc.sync.dma_start(out[bb * 128 : (bb + 1) * 128, :], ys[:])

            # layer 1: B-half dest groups first so AG_B can start early
            for gg in range(ga, g):
                do_phase_a(gg, x1a[:, :], 0, 0)
            for gg in range(ga, g):
                do_phase_b(gg, x1b[:, :], 0, 1, True)
            if split:
                nc.gpsimd.collective_compute(
                    "AllGather",
                    mybir.AluOpType.bypass,
                    replica_groups=[list(range(P))],
                    ins=[x2own_b[:]],
                    outs=[x2hi[:]],
                )
            for gg in range(ga):
                do_phase_a(gg, x1a[:, :], 0, 0)
            for gg in range(ga):
                do_phase_b(gg, x1b[:, :], 0, 1, True)
            nc.gpsimd.collective_compute(
                "AllGather",
                mybir.AluOpType.bypass,
                replica_groups=[list(range(P))],
                ins=[x2own_a[:]],
                outs=[x2lo[:]],
            )
            # layer 2: hi phase first (needs only AG_B), lo merge phase after AG_A
            for gg in range(g):
                do_phase_a(gg, x2hi[:, :], 1, 1)
            for gg in range(g):
                do_phase_b(gg, x2lo[:, :], 1, 0, False)

    nc.finalize()
    return nc


def make_in_maps(pl, x, w1, b1, w2, b2):
    n = x.shape[0]
    npad = pl["sizes"][1]
    x_pad = np.zeros((npad, D), dtype=np.float32)
    x_pad[:n] = x
    shared = {
        "x": x_pad,
        "deg_t": pl["deg_t"],
        "W1": np.ascontiguousarray(w1.astype(np.float32)),
        "b1": np.ascontiguousarray(b1.astype(np.float32).reshape(D, 1)),
        "W2": np.ascontiguousarray(w2.astype(np.float32)),
        "b2": np.ascontiguousarray(b2.astype(np.float32).reshape(D, 1)),
        "b2_tile": np.ascontiguousarray(
            np.tile(b2.astype(np.float32).reshape(1, D), (D, 1))
        ),
        "iota_rep": pl["iota_rep"],
        "ident_bf": pl["ident_bf"],
        "ident_f32": pl["ident_f32"],
    }
    in_maps = []
    for c in range(P):
        m = dict(shared)
        pc = pl["per_core"]
        for kk in ("deg_own", "idx_lo1", "idx_hi1", "idx_lo2", "idx_hi2",
                   "drel1", "drel2"):
            m[kk] = pc[c][kk]
        in_maps.append(m)
    return in_maps


_CACHE = {}


def kernel(x, edge_index, W1, b1, W2, b2):
    from concourse.bass_utils import run_bass_kernel_spmd

    x = np.asarray(x)
    edge_index = np.asarray(edge_index)
    n = x.shape[0]
    pl = plan(edge_index, n)
    key = pl["sizes"]
    if key not in _CACHE:
        _CACHE[key] = build_program(pl)
    nc = _CACHE[key]
    in_maps = make_in_maps(
        pl, x, np.asarray(W1), np.asarray(b1), np.asarray(W2), np.asarray(b2)
    )
    r = run_bass_kernel_spmd(nc, in_maps, list(range(P)))
    outs = np.concatenate([r.results[c]["out"] for c in range(P)], axis=0)
    return np.ascontiguousarray(outs[pl["perm_row"][:n]]).astype(np.float32)
